# revision 41
# baseline (speedup 1.0000x reference)
"""DIN-attention Trainium2 kernel.

out[b] = softmax_t(MLP(concat[q, k, q-k, q*k]) / sqrt(H), mask=t<len_b) @ keys[b]

Strategy (8-core data parallel over B, one shared SPMD program):
- Host sorts b by keys_length, deals round-robin to cores -> per-core slot s
  holds similar lengths on every core; per 16-slot sub-block, work is
  truncated to the sub-block max length (halves all work in expectation).
- MLP decomposition: din@W1 = k@(Wk + diag(q)Wqk) + (Wq^T q + b1) with
  Wk=W1b-W1c, Wqk=W1d, Wq=W1a+W1c; the per-slot fused weight
  Ws = Wk + diag(q_s)Wqk is packed on the host, so m1 is one matmul per
  slot; the per-slot bias (Wq^T q_s + b1) enters the PSUM accumulation
  group through a small K=ns+1 selector matmul pinned to PE row-group 3
  (tile_position=(96,0)), which runs concurrently with the m2 pair
  (K=80, row-groups 0-2) on disjoint PE sub-arrays.
- m2 chunks are emitted in pairs packed at col-groups (0,0)/(0,64) for
  2x PE concurrency; m3 is a single M=1 matmul per chunk packed at PSUM
  partition groups {0,32,64,96} via tile_position, bank-batched.
- Deferred-emission software pipeline (bias/relu1 -> m2 pair/relu2 -> m3
  banks) keeps every matmul's dependencies satisfied before it reaches the
  PE queue head; group g's transpose + output contraction are emitted
  inside group g+1's first sub-block.
- Output contraction runs attn columns (PE-transposed, bf16) as M=1
  stationaries against native-layout bf16 keys with tile_position cycling
  (4x concurrency), accumulated across 128-row chunks over a pre-zeroed
  PSUM bank.
"""

import os
import sys
from contextlib import ExitStack

for _p in ("/opt/trn_rl_repo",):
    if _p not in sys.path:
        sys.path.insert(0, _p)

os.environ.setdefault("CONCOURSE_ENABLE_LDW_OPT", "false")

import numpy as np
import ml_dtypes

import concourse.bass as bass
import concourse.tile as tile
from concourse import bacc, mybir
from concourse.masks import make_identity

F32 = mybir.dt.float32
BF16 = mybir.dt.bfloat16
A = mybir.AluOpType
AF = mybir.ActivationFunctionType

B, T, H = 2048, 200, 128
H1, H2 = 80, 40
NC = 8
SLOTS = B // NC          # 256 slots per core
SB = 16                  # slots per sub-block
NSB = SLOTS // SB        # 16 sub-blocks per core
GROUP_SBS = 4            # sub-blocks per softmax group
NGROUPS = NSB // GROUP_SBS
GSLOTS = GROUP_SBS * SB  # 64 slots per group
SCALE = float(1.0 / np.sqrt(np.float32(H)))
NEG = -1e9


def _roundup(x, m):
    return ((int(x) + m - 1) // m) * m


def _ns_of(tsb):
    # uniform power-of-two slots per chunk with ns*tsb <= 512
    for ns in (16, 8, 4, 2):
        if ns * tsb <= 512:
            return ns
    return 1


def make_plan(keys_length):
    """Global plan shared by all cores: slot assignment + per-sub-block T."""
    order = np.argsort(keys_length, kind="stable")
    bmap = order.reshape(SLOTS, NC)          # [slot, core] -> b
    t_sbs = []
    for sb in range(NSB):
        lens = np.asarray(keys_length)[bmap[sb * SB:(sb + 1) * SB]]
        m = int(lens.max())
        t_sbs.append(min(T, _roundup(m, 8)))
    nchs = [max(1, -(-t // 128)) for t in t_sbs]
    ns_sbs = [_ns_of(t) for t in t_sbs]
    kt_offs, off = [], 0
    for t in t_sbs:
        kt_offs.append(off)
        off += SB * t
    kt_w = off
    # kn: flat, per sub-block, grouped by 128-row chunk; only valid key rows
    # are stored/transferred (cl rows per chunk) to cut HBM traffic.
    kn_cls = [[min(128, t - 128 * c) for c in range(nchs[i])]
              for i, t in enumerate(t_sbs)]
    kn_offs, off = [], 0
    for sb in range(NSB):
        kn_offs.append(off)
        off += sum(kn_cls[sb]) * SB * H
    kn_w = off
    tgs = [max(t_sbs[g * GROUP_SBS:(g + 1) * GROUP_SBS]) for g in range(NGROUPS)]
    # canonical chunk order (must match build_body's emission loops)
    chunks = []
    for sb in range(NSB):
        ns = ns_sbs[sb]
        for ci in range(SB // ns):
            chunks.append((sb, ci, ci * ns, ns, t_sbs[sb]))
    tsb_vals = sorted(set(t_sbs))
    return dict(bmap=bmap, t_sbs=t_sbs, nchs=nchs, ns_sbs=ns_sbs,
                kt_offs=kt_offs, kt_w=kt_w, kn_offs=kn_offs, kn_w=kn_w,
                kn_cls=kn_cls, tgs=tgs, chunks=chunks, tsb_vals=tsb_vals)


def _out_row_in_group(isb, ssb):
    # row of slot ssb (of sub-block isb) within the group's packed out DMA:
    # traversal order (pgroup, isb, colblk)
    return (ssb % 4) * 16 + isb * 4 + ssb // 4


SECTION_MARKS = []


def _mark(nc, label):
    SECTION_MARKS.append((len(nc.inst_map), label))


def build_body(ctx, tc, outs, ins, plan):
    nc = tc.nc
    SECTION_MARKS.clear()
    keysT_d, knat_d, bb_d, fb_d, ws_d = ins
    out_d, = outs
    t_sbs, nchs, ns_sbs = plan["t_sbs"], plan["nchs"], plan["ns_sbs"]
    kt_offs, kn_offs, tgs = plan["kt_offs"], plan["kn_offs"], plan["tgs"]
    kn_cls = plan["kn_cls"]

    singles = ctx.enter_context(tc.tile_pool(name="singles", bufs=1))
    kt_pool = ctx.enter_context(tc.tile_pool(name="kt", bufs=2))
    kn_pool = ctx.enter_context(tc.tile_pool(name="kn", bufs=2 * GROUP_SBS))
    ws_pool = ctx.enter_context(tc.tile_pool(name="ws", bufs=3))
    h1_pool = ctx.enter_context(tc.tile_pool(name="h1", bufs=8))
    h2_pool = ctx.enter_context(tc.tile_pool(name="h2", bufs=8))
    scr_pool = ctx.enter_context(tc.tile_pool(name="scr", bufs=6))
    osc_pool = ctx.enter_context(tc.tile_pool(name="osc", bufs=3))
    grp_pool = ctx.enter_context(tc.tile_pool(name="grp", bufs=2))
    at_pool = ctx.enter_context(tc.tile_pool(name="at", bufs=4))
    ps1_pool = ctx.enter_context(tc.tile_pool(name="ps1", bufs=2, space="PSUM"))
    pst_pool = ctx.enter_context(tc.tile_pool(name="pst", bufs=1, space="PSUM"))
    ps2_pool = ctx.enter_context(tc.tile_pool(name="ps2", bufs=2, space="PSUM"))
    psper_pool = ctx.enter_context(tc.tile_pool(name="psper", bufs=1, space="PSUM"))

    # ---- first kt group + first sub-block weights: prefetch before the
    # constants so the opening m1 isn't starved behind small DMAs ----
    ktgs = {}
    kt_pool_ref = kt_pool

    def emit_ktg(g, split=False):
        if g >= NGROUPS or g in ktgs:
            return
        _mark(nc, 'dma_kt')
        kt0_ = kt_offs[g * GROUP_SBS]
        ktg_w = sum(SB * t_sbs[g * GROUP_SBS + i] for i in range(GROUP_SBS))
        t = kt_pool_ref.tile([H, ktg_w], BF16, tag="kt", name=f"ktg_{g}")
        if split:
            # three transfers: sb0 | sb1 | sb2+sb3, so the opening m1 can
            # begin as soon as the first (small) slice has landed
            w0 = SB * t_sbs[g * GROUP_SBS]
            w1 = w0 + SB * t_sbs[g * GROUP_SBS + 1]
            nc.sync.dma_start(t[:, 0:w0], keysT_d[:, kt0_:kt0_ + w0])
            nc.sync.dma_start(t[:, w0:w1], keysT_d[:, kt0_ + w0:kt0_ + w1])
            nc.sync.dma_start(t[:, w1:ktg_w],
                              keysT_d[:, kt0_ + w1:kt0_ + ktg_w])
        else:
            nc.sync.dma_start(t[:], keysT_d[:, kt0_:kt0_ + ktg_w])
        ktgs[g] = t

    GORDER = [1, 2, 3, 0]
    SB_ORDER = [g_ * GROUP_SBS + i_ for g_ in GORDER for i_ in range(GROUP_SBS)]
    # startup order matters: each DMA trigger occupies the queue ~0.6-0.9us,
    # so the first m1's dependencies (kt sb0 slice, const bundles, wss0) are
    # triggered FIRST, in 4 transfers total.
    sbf = SB_ORDER[0]
    kt0_ = kt_offs[sbf]
    w0_ = SB * t_sbs[sbf]
    ktg_w0 = sum(SB * t_sbs[sbf + i] for i in range(GROUP_SBS))
    ktg0 = kt_pool.tile([H, ktg_w0], BF16, tag="kt", name=f"ktg_{GORDER[0]}")
    nc.sync.dma_start(ktg0[:, 0:w0_], keysT_d[:, kt0_:kt0_ + w0_])
    wss_first = ws_pool.tile([H, SB * H1], BF16, tag="wss", name=f"wss_{sbf}")
    nc.sync.dma_start(wss_first[:], ws_d[:, sbf * SB * H1:(sbf + 1) * SB * H1])
    # bb: [qt | wq | wf2 | w2] bf16 bundle, fb: [b1 | b2(x2) | lens] f32 bundle
    bb = singles.tile([H, SLOTS + H1 + 1 + H2], BF16, name="bb")
    nc.sync.dma_start(bb[:], bb_d)
    fb = singles.tile([128, 2 + NGROUPS], F32, name="fb")
    nc.sync.dma_start(fb[:], fb_d)
    qt = bb[:, 0:SLOTS]
    wq = bb[:, SLOTS:SLOTS + H1]
    wf2 = bb[:, SLOTS + H1:SLOTS + H1 + 1]
    w2 = bb[0:H1, SLOTS + H1 + 1:SLOTS + H1 + 1 + H2]
    b1c = fb[0:H1, 0:1]
    b2c = fb[0:H2, 1:2]
    b2c2 = fb[0:64 + H2, 1:2]
    lens = fb[0:GSLOTS, 2:2 + NGROUPS]
    wss_pre = {sbf: wss_first}
    t_ = ws_pool.tile([H, SB * H1], BF16, tag="wss", name=f"wss_{sbf + 1}")
    nc.sync.dma_start(t_[:], ws_d[:, (sbf + 1) * SB * H1:(sbf + 2) * SB * H1])
    wss_pre[sbf + 1] = t_
    # rest of the first group's keysT
    w1_ = w0_ + SB * t_sbs[sbf + 1]
    nc.sync.dma_start(ktg0[:, w0_:w1_], keysT_d[:, kt0_ + w0_:kt0_ + w1_])
    nc.sync.dma_start(ktg0[:, w1_:ktg_w0], keysT_d[:, kt0_ + w1_:kt0_ + ktg_w0])
    ktgs[GORDER[0]] = ktg0
    iota = singles.tile([128, T], F32, name="iota")
    nc.gpsimd.iota(iota[:], pattern=[[1, T]], base=0, channel_multiplier=0,
                   allow_small_or_imprecise_dtypes=True)
    identb = singles.tile([128, 128], BF16, name="identb")
    make_identity(nc, identb[:])
    zeros1 = singles.tile([1, 128], BF16, name="zeros1")
    nc.vector.memset(zeros1[:], 0.0)
    dummy512 = singles.tile([1, 512], BF16, name="dummy512")
    nc.vector.memset(dummy512[:], 0.0)
    # per-group per-slot length masks (t < len), on the otherwise-idle gpsimd
    masks = []
    for g in range(NGROUPS):
        tg = tgs[g]
        mk = singles.tile([GSLOTS, tg], F32, name=f"mask_{g}")
        nc.gpsimd.tensor_scalar(mk[:], iota[0:GSLOTS, 0:tg],
                                fb[0:GSLOTS, 2 + g:3 + g],
                                None, op0=A.is_lt)
        masks.append(mk)
    # persistent, one-time-zeroed PSUM banks
    pss_t = [psper_pool.tile([128, 512], F32, tag=f"pssp{i}", name=f"pssp{i}")
             for i in range(2)]
    pso_t = [psper_pool.tile([128, 512], F32, tag="psop0", name="psop0")] * 2
    for t_ in pss_t + pso_t[:1]:
        nc.tensor.matmul(t_[:], zeros1[:], dummy512[:], start=True, stop=True)


    bb_pitch = bb[:].ap[0][0]
    bb_off = bb[:].offset
    par = [0]      # parity counter for DVE/ACT copy balancing
    bankctr = [0]  # m3 scores-bank alternation
    ckctr = [0]    # global chunk counter

    # deferred-emission pipeline queues. Each m2q/m3q entry is (mm, post):
    # the mm parts of a pair/bank are emitted NEWEST-FIRST so the first
    # matmul's semaphore wait subsumes the older ones — the rest arrive at
    # the PE wait-free and overlap via their disjoint tile_position groups
    # (the same reason the final stage's shared-dep matmuls overlap).
    m2q, m3q = [], []
    finq = []
    zsums = {}
    trans_prev = [None]
    final_prev = [None]

    def pump_m3(force=False):
        while m3q:
            key = m3q[0][0]
            nbank = sum(1 for k, _ in m3q if k == key)
            rest = len(m3q) - nbank
            if rest < 6 and not force:
                return
            units = [m3q.pop(0)[1] for _ in range(nbank)]
            for u in reversed(units):
                u[0]()
            for u in units:
                u[1]()

    def pump(force=False):
        while len(m2q) >= (2 if force else 3):
            ua, ub = m2q.pop(0), m2q.pop(0)
            ps2 = ps2_pool.tile([128, 512], F32, tag="ps2",
                                name=f"ps2p_{ckctr[0]}_{len(m2q)}")
            ub[0](ps2, 64)
            ua[0](ps2, 0)
            maxc = max(ua[2], ub[2])
            _mark(nc, 'relu2')
            h2 = h2_pool.tile([64 + H2, 512], BF16, tag="h2",
                              name=f"h2p_{ckctr[0]}_{len(m2q)}")
            nc.scalar.activation(h2[:, 0:maxc], ps2[0:64 + H2, 0:maxc],
                                 AF.Relu, bias=b2c2, scale=1.0)
            ua[1](h2, 0)
            ub[1](h2, 64)
        if force and m2q:
            ps2 = ps2_pool.tile([128, 512], F32, tag="ps2", name="ps2last")
            u = m2q.pop(0)
            u[0](ps2, 0)
            _mark(nc, 'relu2')
            h2 = h2_pool.tile([64 + H2, 512], BF16, tag="h2", name="h2last")
            nc.scalar.activation(h2[0:H2, 0:u[2]], ps2[0:H2, 0:u[2]],
                                 AF.Relu, bias=b2c, scale=1.0)
            u[1](h2, 0)
        pump_m3(force)

    for gi, g in enumerate(GORDER):
        tg = tgs[g]
        scores = grp_pool.tile([GSLOTS, tg], F32, tag="scores", name=f"scores_g{g}")
        _mark(nc, 'memset')
        nc.gpsimd.memset(scores[:], 0.0)
        emit_ktg(g)
        ktg = ktgs.pop(g)
        kt0 = kt_offs[g * GROUP_SBS]
        knats = {}
        for isb in range(GROUP_SBS):
            sb = g * GROUP_SBS + isb
            tsb, nch, ns = t_sbs[sb], nchs[sb], ns_sbs[sb]
            nchunks = SB // ns
            kto = kt_offs[sb] - kt0
            wss = wss_pre.pop(sb)
            # prefetch the fused weights two sub-blocks ahead so the m1
            # stream never waits on the ws DMA
            pos_ = SB_ORDER.index(sb)
            for sbn in SB_ORDER[pos_ + 1:pos_ + 3]:
                if sbn not in wss_pre:
                    _mark(nc, 'dma_ws')
                    t_ = ws_pool.tile([H, SB * H1], BF16, tag="wss",
                                      name=f"wss_{sbn}")
                    nc.sync.dma_start(t_[:],
                                      ws_d[:, sbn * SB * H1:(sbn + 1) * SB * H1])
                    wss_pre[sbn] = t_
            _mark(nc, 'dma_kn')
            kn = kn_pool.tile([128, SB * nch * 128], BF16, tag="kn", name=f"kn_{sb}")
            o_ = kn_offs[sb]
            for c_, cl_ in enumerate(kn_cls[sb]):
                src = bass.AP(tensor=knat_d.tensor, offset=o_,
                              ap=[[SB * H, cl_], [1, SB * H]])
                nc.sync.dma_start(kn[0:cl_, c_ * SB * H:(c_ + 1) * SB * H], src)
                o_ += cl_ * SB * H
            knats[sb] = (kn, nch)

            for bank in range(-(-nchunks // 4)):
                cis = range(4 * bank, min(nchunks, 4 * bank + 4))
                cols = ns * tsb
                for ci in cis:
                    s0 = ci * ns
                    coff = kto + s0 * tsb
                    ckctr[0] += 1
                    _mark(nc, 'm1')
                    ps1 = ps1_pool.tile([H1, cols], F32, tag="ps1",
                                        name=f"ps1_{sb}_{ci}")
                    for j in range(ns):
                        si = (s0 + j) * H1
                        nc.tensor.matmul(ps1[:, j * tsb:(j + 1) * tsb],
                                         wss[:, si:si + H1],
                                         ktg[:, coff + j * tsb:coff + (j + 1) * tsb],
                                         start=(j == 0), stop=False,
                                         skip_group_check=True)
                    qview = bass.AP(tensor=bb[:].tensor,
                                    offset=bb_off + sb * SB + s0,
                                    ap=[[bb_pitch, H], [1, ns], [0, tsb]])
                    nc.tensor.matmul(ps1[:, 0:cols], wq, qview,
                                     start=False, stop=True,
                                     skip_group_check=True)
                    _mark(nc, 'relu1')
                    h1 = h1_pool.tile([H1, cols], BF16, tag="h1",
                                      name=f"h1_{sb}_{ci}")
                    if ckctr[0] % 3 == 0:
                        nc.scalar.activation(h1[:], ps1[:], AF.Relu,
                                             bias=b1c, scale=1.0)
                    else:
                        nc.vector.tensor_scalar(h1[:], ps1[:], b1c, 0.0,
                                                op0=A.add, op1=A.max)

                    def m2_mm(ps2, base, cols=cols, h1=h1):
                        _mark(nc, 'm2')
                        nc.tensor.matmul(ps2[base:base + H2, 0:cols], w2,
                                         h1[:], start=True, stop=True,
                                         tile_position=(0, base),
                                         skip_group_check=True)

                    def m2_post(h2, rowbase, sb=sb, isb=isb, bank=bank, ci=ci,
                                cols=cols, tsb=tsb, ns=ns,
                                last=(ci == cis[-1]), npg=len(cis),
                                scores=scores):
                        def m3_mm():
                            _mark(nc, 'm3')
                            pg = ci % 4
                            pss = pss_t[bankctr[0] % 2]
                            nc.tensor.matmul(pss[32 * pg:32 * pg + 1, 0:cols],
                                             bb[rowbase:rowbase + H2,
                                                SLOTS + H1:SLOTS + H1 + 1],
                                             h2[rowbase:rowbase + H2, 0:cols],
                                             start=True, stop=True,
                                             tile_position=(rowbase, 32 * pg),
                                             skip_group_check=True)

                        def m3_post():
                            if not last:
                                return
                            pss = pss_t[bankctr[0] % 2]
                            bankctr[0] += 1
                            _mark(nc, 'scr_copy')
                            scratch = scr_pool.tile([128, 512], F32, tag="scr",
                                                    name=f"scr_{sb}_{bank}")
                            nc.scalar.activation(scratch[:, 0:cols],
                                                 pss[:, 0:cols], AF.Exp,
                                                 bias=0.0, scale=SCALE)
                            _mark(nc, 'redis_s')
                            scr_pitch = scratch[:].ap[0][0]
                            src = bass.AP(tensor=scratch[:].tensor,
                                          offset=scratch[:].offset,
                                          ap=[[32 * scr_pitch, npg],
                                              [tsb, ns], [1, tsb]])
                            r0 = 16 * isb + 4 * bank * ns
                            nc.gpsimd.dma_start(
                                scores[r0:r0 + npg * ns, 0:tsb], src)

                        m3q.append(((sb, bank), (m3_mm, m3_post)))

                    m2q.append((m2_mm, m2_post, cols))
                    pump()
            if isb == 0 and finq:
                finq.pop(0)()
            if isb == 1:
                if finq:
                    finq.pop(0)()
                if gi + 1 < NGROUPS:
                    emit_ktg(GORDER[gi + 1])
            if isb == 2:
                if trans_prev[0] is not None:
                    trans_prev[0]()
                    trans_prev[0] = None
                if final_prev[0] is not None:
                    finq.extend(final_prev[0]())
                    final_prev[0] = None
                for _ in range(2):
                    if finq:
                        finq.pop(0)()
            if isb == 3:
                for _ in range(2):
                    if finq:
                        finq.pop(0)()
        pump(force=True)

        atts = []

        def trans_unit(g=g, tg=tg, scores=scores, atts=atts):
            # softmax mostly on the otherwise-idle gpsimd so DVE/ACT queues
            # stay clear for the relu stream
            _mark(nc, 'softmax')
            pm = grp_pool.tile([GSLOTS, tg], F32, tag="pm", name=f"pm_{g}")
            nc.vector.tensor_tensor(pm[:], scores[:], masks[g][:], op=A.mult)
            zsum = grp_pool.tile([GSLOTS, 1], F32, tag="zsum", name=f"zsum_{g}")
            nc.vector.reduce_sum(zsum[:], pm[:], axis=mybir.AxisListType.X)
            rz = grp_pool.tile([GSLOTS, 1], F32, tag="rz", name=f"rz_{g}")
            nc.vector.reciprocal(rz[:], zsum[:])
            attnb = grp_pool.tile([GSLOTS, tg], BF16, tag="attnb",
                                  name=f"attnb_{g}")
            nc.vector.tensor_scalar_mul(attnb[:], pm[:], rz[:, 0:1])
            _mark(nc, 'transpose')
            for c in range(-(-tg // 128)):
                cl = min(128, tg - 128 * c)
                ps_t = pst_pool.tile([cl, GSLOTS], BF16, tag="pst",
                                     name=f"pst_{g}_{c}")
                nc.tensor.transpose(ps_t[:], attnb[:, 128 * c:128 * c + cl],
                                    identb[0:GSLOTS, 0:GSLOTS])
                at = at_pool.tile([cl, GSLOTS], BF16, tag="at", name=f"at_{g}_{c}")
                nc.vector.tensor_copy(at[:], ps_t[:])
                atts.append(at)

        def make_final_units(g=g, knats=knats, atts=atts):
            oscr_box = [None]

            def f_sb(isb):
                def run():
                    _mark(nc, 'final')
                    if oscr_box[0] is None:
                        oscr_box[0] = osc_pool.tile(
                            [128, GROUP_SBS * 512], F32, tag="oscr",
                            name=f"oscr_{g}")
                    oscr = oscr_box[0]
                    sb = g * GROUP_SBS + isb
                    tsb = t_sbs[sb]
                    kn, nch = knats[sb]
                    ps_o = pso_t[sb % 2]
                    if nch == 1:
                        for ssb in range(SB):
                            r = 16 * isb + ssb
                            cl = tsb
                            blk = ssb * 128
                            nc.tensor.matmul(
                                ps_o[32 * (ssb % 4):32 * (ssb % 4) + 1,
                                     128 * (ssb // 4):128 * (ssb // 4) + 128],
                                atts[0][0:cl, r:r + 1], kn[0:cl, blk:blk + 128],
                                start=True, stop=True,
                                tile_position=(0, 32 * (ssb % 4)),
                                skip_group_check=True)
                    else:
                        nc.tensor.matmul(ps_o[:], zeros1[:], dummy512[:],
                                         start=True, stop=False,
                                         skip_group_check=True)
                        for c in range(nch):
                            cl = min(128, tsb - 128 * c)
                            for ssb in range(SB):
                                r = 16 * isb + ssb
                                blk = c * SB * H + ssb * 128
                                nc.tensor.matmul(
                                    ps_o[32 * (ssb % 4):32 * (ssb % 4) + 1,
                                         128 * (ssb // 4):128 * (ssb // 4) + 128],
                                    atts[c][0:cl, r:r + 1],
                                    kn[0:cl, blk:blk + 128],
                                    start=False, stop=(c == nch - 1),
                                    tile_position=(0, 32 * (ssb % 4)),
                                    skip_group_check=True)
                    _mark(nc, 'oscr_copy')
                    nc.vector.tensor_copy(oscr[:, isb * 512:isb * 512 + 512],
                                          ps_o[:])
                return run

            def f_out():
                _mark(nc, 'out_dma')
                oscr = oscr_box[0]
                os_pitch = oscr[:].ap[0][0]
                src = bass.AP(tensor=oscr[:].tensor, offset=oscr[:].offset,
                              ap=[[32 * os_pitch, 4], [512, 4], [128, 4],
                                  [1, 128]])
                nc.sync.dma_start(out_d[g * GSLOTS:(g + 1) * GSLOTS, :], src)

            return [f_sb(0), f_sb(1), f_sb(2), f_sb(3), f_out]

        trans_prev[0] = trans_unit
        final_prev[0] = make_final_units
    trans_prev[0]()
    for u in final_prev[0]():
        u()
    while finq:
        finq.pop(0)()


def pack_inputs(query, keys, keys_length, W1, b1, W2, b2, Wf, bf, plan):
    """Build the 8 per-core input maps."""
    bmap, t_sbs, nchs = plan["bmap"], plan["t_sbs"], plan["nchs"]
    kt_w, kn_w = plan["kt_w"], plan["kn_w"]
    Wq = (W1[0:H] + W1[2 * H:3 * H]).astype(np.float32)
    Wk = (W1[H:2 * H] - W1[2 * H:3 * H]).astype(np.float32)
    Wqk = W1[3 * H:4 * H].astype(np.float32)
    bfl = ml_dtypes.bfloat16
    in_maps = []
    for c in range(NC):
        ktp = np.zeros((H, kt_w), bfl)
        knp = np.zeros((kn_w,), bfl)
        qtp = np.zeros((H, SLOTS), np.float32)
        lensp = np.zeros((GSLOTS, NGROUPS), np.float32)
        for sb in range(NSB):
            tsb, nch = t_sbs[sb], nchs[sb]
            ko, no = plan["kt_offs"][sb], plan["kn_offs"][sb]
            g, isb = sb // GROUP_SBS, sb % GROUP_SBS
            for ssb in range(SB):
                s = sb * SB + ssb
                b = int(bmap[s, c])
                ktp[:, ko + ssb * tsb: ko + (ssb + 1) * tsb] = keys[b, :tsb, :].T
                o_ = no
                for ch, cl in enumerate(plan["kn_cls"][sb]):
                    view = knp[o_:o_ + cl * SB * H].reshape(cl, SB, H)
                    view[:, ssb, :] = keys[b, 128 * ch:128 * ch + cl, :]
                    o_ += cl * SB * H
                qtp[:, s] = query[b]
                lensp[16 * isb + ssb, g] = keys_length[b]
        # per-slot fused first-layer weights: Ws = Wk + diag(q_s) Wqk
        wsp = (Wk[:, None, :] + qtp[:, :, None] * Wqk[:, None, :])  # [H,SLOTS,H1]
        wsp = wsp.reshape(H, SLOTS * H1).astype(bfl)
        # bf16 const bundle: [qt | wq | wf(x2 bands) | w2]
        bbp = np.zeros((H, SLOTS + H1 + 1 + H2), bfl)
        bbp[:, 0:SLOTS] = qtp
        bbp[:, SLOTS:SLOTS + H1] = Wq
        bbp[0:H2, SLOTS + H1] = Wf[:, 0]
        bbp[64:64 + H2, SLOTS + H1] = Wf[:, 0]
        bbp[0:H1, SLOTS + H1 + 1:] = W2
        # f32 const bundle: [b1 | b2(x2 bands) | lens]
        fbp = np.zeros((128, 2 + NGROUPS), np.float32)
        fbp[0:H1, 0] = b1
        fbp[0:H2, 1] = b2
        fbp[64:64 + H2, 1] = b2
        fbp[0:GSLOTS, 2:] = lensp
        in_maps.append({"keysT": ktp, "knat": knp, "bb": bbp, "fb": fbp,
                        "ws": wsp})
    return in_maps


def build_program(plan):
    nc = bacc.Bacc("TRN2", num_devices=NC)
    ins = [
        nc.dram_tensor("keysT", [H, plan["kt_w"]], BF16, kind="ExternalInput").ap(),
        nc.dram_tensor("knat", [plan["kn_w"]], BF16, kind="ExternalInput").ap(),
        nc.dram_tensor("bb", [H, SLOTS + H1 + 1 + H2], BF16,
                       kind="ExternalInput").ap(),
        nc.dram_tensor("fb", [128, 2 + NGROUPS], F32, kind="ExternalInput").ap(),
        nc.dram_tensor("ws", [H, SLOTS * H1], BF16, kind="ExternalInput").ap(),
    ]
    outs = [nc.dram_tensor("outN", [SLOTS, H], F32, kind="ExternalOutput").ap()]
    with tile.TileContext(nc) as tc:
        with ExitStack() as ctx:
            build_body(ctx, tc, outs, ins, plan)
    nc.compile()
    return nc


last_results = None  # stash for external profiling/analysis


def kernel(query, keys, keys_length, W1, b1, W2, b2, Wf, bf):
    global last_results
    from concourse.bass_utils import run_bass_kernel_spmd
    query = np.asarray(query, np.float32)
    keys = np.asarray(keys, np.float32)
    keys_length = np.asarray(keys_length)
    plan = make_plan(keys_length)
    in_maps = pack_inputs(query, keys, keys_length, np.asarray(W1, np.float32),
                          np.asarray(b1, np.float32), np.asarray(W2, np.float32),
                          np.asarray(b2, np.float32), np.asarray(Wf, np.float32),
                          np.asarray(bf, np.float32), plan)
    nc = build_program(plan)
    trace = bool(int(os.environ.get("BASS_KERNEL_TRACE", "0")))
    res = run_bass_kernel_spmd(nc, in_maps, core_ids=list(range(NC)), trace=trace)
    last_results = res
    globals()["last_nc"] = nc
    if trace and res.exec_time_ns is not None:
        print(f"HW exec time: {res.exec_time_ns} ns")
    out = np.zeros((B, H), np.float32)
    bmap = plan["bmap"]
    rows = np.array([GSLOTS * (s // GSLOTS)
                     + _out_row_in_group((s // SB) % GROUP_SBS, s % SB)
                     for s in range(SLOTS)])
    for c in range(NC):
        outN = res.results[c]["outN"]  # [SLOTS, H]
        out[bmap[:, c]] = outN[rows]
    return out



# revision 44
# speedup vs baseline: 1.0192x; 1.0192x over previous
"""DIN-attention Trainium2 kernel.

out[b] = softmax_t(MLP(concat[q, k, q-k, q*k]) / sqrt(H), mask=t<len_b) @ keys[b]

Strategy (8-core data parallel over B, one shared SPMD program):
- Host sorts b by keys_length, deals round-robin to cores -> per-core slot s
  holds similar lengths on every core; per 16-slot sub-block, work is
  truncated to the sub-block max length (halves all work in expectation).
- MLP decomposition: din@W1 = k@(Wk + diag(q)Wqk) + (Wq^T q + b1) with
  Wk=W1b-W1c, Wqk=W1d, Wq=W1a+W1c; the per-slot fused weight
  Ws = Wk + diag(q_s)Wqk is packed on the host, so m1 is one matmul per
  slot; the per-slot bias (Wq^T q_s + b1) enters the PSUM accumulation
  group through a small K=ns+1 selector matmul pinned to PE row-group 3
  (tile_position=(96,0)), which runs concurrently with the m2 pair
  (K=80, row-groups 0-2) on disjoint PE sub-arrays.
- m2 chunks are emitted in pairs packed at col-groups (0,0)/(0,64) for
  2x PE concurrency; m3 is a single M=1 matmul per chunk packed at PSUM
  partition groups {0,32,64,96} via tile_position, bank-batched.
- Deferred-emission software pipeline (bias/relu1 -> m2 pair/relu2 -> m3
  banks) keeps every matmul's dependencies satisfied before it reaches the
  PE queue head; group g's transpose + output contraction are emitted
  inside group g+1's first sub-block.
- Output contraction runs attn columns (PE-transposed, bf16) as M=1
  stationaries against native-layout bf16 keys with tile_position cycling
  (4x concurrency), accumulated across 128-row chunks over a pre-zeroed
  PSUM bank.
"""

import os
import sys
from contextlib import ExitStack

for _p in ("/opt/trn_rl_repo",):
    if _p not in sys.path:
        sys.path.insert(0, _p)

os.environ.setdefault("CONCOURSE_ENABLE_LDW_OPT", "false")

import numpy as np
import ml_dtypes

import concourse.bass as bass
import concourse.tile as tile
from concourse import bacc, mybir
from concourse.masks import make_identity

F32 = mybir.dt.float32
BF16 = mybir.dt.bfloat16
A = mybir.AluOpType
AF = mybir.ActivationFunctionType

B, T, H = 2048, 200, 128
H1, H2 = 80, 40
NC = 8
SLOTS = B // NC          # 256 slots per core
SB = 16                  # slots per sub-block
NSB = SLOTS // SB        # 16 sub-blocks per core
GROUP_SBS = 4            # sub-blocks per softmax group
NGROUPS = NSB // GROUP_SBS
GSLOTS = GROUP_SBS * SB  # 64 slots per group
SCALE = float(1.0 / np.sqrt(np.float32(H)))
NEG = -1e9


def _roundup(x, m):
    return ((int(x) + m - 1) // m) * m


def _ns_of(tsb):
    # uniform power-of-two slots per chunk with ns*tsb <= 512
    for ns in (16, 8, 4, 2):
        if ns * tsb <= 512:
            return ns
    return 1


def make_plan(keys_length):
    """Global plan shared by all cores: slot assignment + per-sub-block T."""
    order = np.argsort(keys_length, kind="stable")
    bmap = order.reshape(SLOTS, NC)          # [slot, core] -> b
    t_sbs = []
    for sb in range(NSB):
        lens = np.asarray(keys_length)[bmap[sb * SB:(sb + 1) * SB]]
        m = int(lens.max())
        t_sbs.append(min(T, _roundup(m, 8)))
    nchs = [max(1, -(-t // 128)) for t in t_sbs]
    ns_sbs = [_ns_of(t) for t in t_sbs]
    kt_offs, off = [], 0
    for t in t_sbs:
        kt_offs.append(off)
        off += SB * t
    kt_w = off
    # kn: flat, per sub-block, grouped by 128-row chunk; only valid key rows
    # are stored/transferred (cl rows per chunk) to cut HBM traffic.
    kn_cls = [[min(128, t - 128 * c) for c in range(nchs[i])]
              for i, t in enumerate(t_sbs)]
    kn_offs, off = [], 0
    for sb in range(NSB):
        kn_offs.append(off)
        off += sum(kn_cls[sb]) * SB * H
    kn_w = off
    tgs = [max(t_sbs[g * GROUP_SBS:(g + 1) * GROUP_SBS]) for g in range(NGROUPS)]
    # canonical chunk order (must match build_body's emission loops)
    chunks = []
    for sb in range(NSB):
        ns = ns_sbs[sb]
        for ci in range(SB // ns):
            chunks.append((sb, ci, ci * ns, ns, t_sbs[sb]))
    tsb_vals = sorted(set(t_sbs))
    pat_offs = {}
    sel_w = NSB * H2 + SB
    return dict(bmap=bmap, pat_offs=pat_offs, sel_w=sel_w, t_sbs=t_sbs, nchs=nchs, ns_sbs=ns_sbs,
                kt_offs=kt_offs, kt_w=kt_w, kn_offs=kn_offs, kn_w=kn_w,
                kn_cls=kn_cls, tgs=tgs, chunks=chunks, tsb_vals=tsb_vals)


def _out_row_in_group(isb, ssb):
    # row of slot ssb (of sub-block isb) within the group's packed out DMA:
    # traversal order (pgroup, isb, colblk)
    return (ssb % 4) * 16 + isb * 4 + ssb // 4


SECTION_MARKS = []


def _mark(nc, label):
    SECTION_MARKS.append((len(nc.inst_map), label))


def build_body(ctx, tc, outs, ins, plan):
    nc = tc.nc
    SECTION_MARKS.clear()
    keysT_d, knat_d, bb_d, fb_d, ws_d, sel_d, selw = ins
    out_d, = outs
    t_sbs, nchs, ns_sbs = plan["t_sbs"], plan["nchs"], plan["ns_sbs"]
    kt_offs, kn_offs, tgs = plan["kt_offs"], plan["kn_offs"], plan["tgs"]
    kn_cls = plan["kn_cls"]
    pat_offs = plan["pat_offs"]

    singles = ctx.enter_context(tc.tile_pool(name="singles", bufs=1))
    kt_pool = ctx.enter_context(tc.tile_pool(name="kt", bufs=2))
    kn_pool = ctx.enter_context(tc.tile_pool(name="kn", bufs=2 * GROUP_SBS))
    ws_pool = ctx.enter_context(tc.tile_pool(name="ws", bufs=3))
    h1_pool = ctx.enter_context(tc.tile_pool(name="h1", bufs=8))
    h2_pool = ctx.enter_context(tc.tile_pool(name="h2", bufs=8))
    scr_pool = ctx.enter_context(tc.tile_pool(name="scr", bufs=6))
    osc_pool = ctx.enter_context(tc.tile_pool(name="osc", bufs=3))
    grp_pool = ctx.enter_context(tc.tile_pool(name="grp", bufs=2))
    at_pool = ctx.enter_context(tc.tile_pool(name="at", bufs=4))
    ps1_pool = ctx.enter_context(tc.tile_pool(name="ps1", bufs=2, space="PSUM"))
    pst_pool = ctx.enter_context(tc.tile_pool(name="pst", bufs=1, space="PSUM"))
    ps2_pool = ctx.enter_context(tc.tile_pool(name="ps2", bufs=2, space="PSUM"))
    psper_pool = ctx.enter_context(tc.tile_pool(name="psper", bufs=1, space="PSUM"))

    # ---- first kt group + first sub-block weights: prefetch before the
    # constants so the opening m1 isn't starved behind small DMAs ----
    ktgs = {}
    kt_pool_ref = kt_pool

    def emit_ktg(g, split=False):
        if g >= NGROUPS or g in ktgs:
            return
        _mark(nc, 'dma_kt')
        kt0_ = kt_offs[g * GROUP_SBS]
        ktg_w = sum(SB * t_sbs[g * GROUP_SBS + i] for i in range(GROUP_SBS))
        t = kt_pool_ref.tile([H, ktg_w], BF16, tag="kt", name=f"ktg_{g}")
        if split:
            # three transfers: sb0 | sb1 | sb2+sb3, so the opening m1 can
            # begin as soon as the first (small) slice has landed
            w0 = SB * t_sbs[g * GROUP_SBS]
            w1 = w0 + SB * t_sbs[g * GROUP_SBS + 1]
            nc.sync.dma_start(t[:, 0:w0], keysT_d[:, kt0_:kt0_ + w0])
            nc.sync.dma_start(t[:, w0:w1], keysT_d[:, kt0_ + w0:kt0_ + w1])
            nc.sync.dma_start(t[:, w1:ktg_w],
                              keysT_d[:, kt0_ + w1:kt0_ + ktg_w])
        else:
            nc.sync.dma_start(t[:], keysT_d[:, kt0_:kt0_ + ktg_w])
        ktgs[g] = t

    GORDER = [1, 2, 3, 0]
    SB_ORDER = [g_ * GROUP_SBS + i_ for g_ in GORDER for i_ in range(GROUP_SBS)]
    # startup order matters: each DMA trigger occupies the queue ~0.6-0.9us,
    # so the first m1's dependencies (kt sb0 slice, const bundles, wss0) are
    # triggered FIRST, in 4 transfers total.
    sbf = SB_ORDER[0]
    kt0_ = kt_offs[sbf]
    w0_ = SB * t_sbs[sbf]
    ktg_w0 = sum(SB * t_sbs[sbf + i] for i in range(GROUP_SBS))
    ktg0 = kt_pool.tile([H, ktg_w0], BF16, tag="kt", name=f"ktg_{GORDER[0]}")
    nc.sync.dma_start(ktg0[:, 0:w0_], keysT_d[:, kt0_:kt0_ + w0_])
    wss_first = ws_pool.tile([H, SB * H1], BF16, tag="wss", name=f"wss_{sbf}")
    nc.sync.dma_start(wss_first[:], ws_d[:, sbf * SB * H1:(sbf + 1) * SB * H1])
    # bb: [qt | wq | wf2 | w2] bf16 bundle, fb: [b1 | b2(x2) | lens] f32 bundle
    bb = singles.tile([H, SLOTS + H1 + 1 + H2], BF16, name="bb")
    nc.sync.dma_start(bb[:], bb_d)
    fb = singles.tile([128, 2 + NGROUPS + SLOTS], F32, name="fb")
    nc.sync.dma_start(fb[:], fb_d)
    qt = bb[:, 0:SLOTS]
    wq = bb[:, SLOTS:SLOTS + H1]
    wf2 = bb[:, SLOTS + H1:SLOTS + H1 + 1]
    w2 = bb[0:H1, SLOTS + H1 + 1:SLOTS + H1 + 1 + H2]
    b1c = fb[0:H1, 0:1]
    b2c = fb[0:H2, 1:2]
    b2c2 = fb[0:64 + H2, 1:2]
    lens = fb[0:GSLOTS, 2:2 + NGROUPS]
    seltile = singles.tile([128, selw], BF16, name="seltile")
    nc.sync.dma_start(seltile[96:112, :], sel_d)
    sel_pitch = seltile[:].ap[0][0]
    wss_pre = {sbf: wss_first}
    t_ = ws_pool.tile([H, SB * H1], BF16, tag="wss", name=f"wss_{sbf + 1}")
    nc.sync.dma_start(t_[:], ws_d[:, (sbf + 1) * SB * H1:(sbf + 2) * SB * H1])
    wss_pre[sbf + 1] = t_
    # rest of the first group's keysT
    w1_ = w0_ + SB * t_sbs[sbf + 1]
    nc.sync.dma_start(ktg0[:, w0_:w1_], keysT_d[:, kt0_ + w0_:kt0_ + w1_])
    nc.sync.dma_start(ktg0[:, w1_:ktg_w0], keysT_d[:, kt0_ + w1_:kt0_ + ktg_w0])
    ktgs[GORDER[0]] = ktg0
    iota = singles.tile([128, T], F32, name="iota")
    nc.gpsimd.iota(iota[:], pattern=[[1, T]], base=0, channel_multiplier=0,
                   allow_small_or_imprecise_dtypes=True)
    identb = singles.tile([128, 128], BF16, name="identb")
    make_identity(nc, identb[:])
    zeros1 = singles.tile([1, 128], BF16, name="zeros1")
    nc.vector.memset(zeros1[:], 0.0)
    dummy512 = singles.tile([1, 512], BF16, name="dummy512")
    nc.vector.memset(dummy512[:], 0.0)
    # per-group per-slot length masks (t < len), on the otherwise-idle gpsimd
    masks = []
    for g in range(NGROUPS):
        tg = tgs[g]
        mk = singles.tile([GSLOTS, tg], F32, name=f"mask_{g}")
        nc.gpsimd.tensor_scalar(mk[:], iota[0:GSLOTS, 0:tg],
                                fb[0:GSLOTS, 2 + g:3 + g],
                                None, op0=A.is_lt)
        masks.append(mk)
    # persistent, one-time-zeroed PSUM banks
    pss_t = [psper_pool.tile([128, 512], F32, tag=f"pssp{i}", name=f"pssp{i}")
             for i in range(2)]
    pso_t = [psper_pool.tile([128, 512], F32, tag="psop0", name="psop0")] * 2
    for t_ in pss_t + pso_t[:1]:
        nc.tensor.matmul(t_[:], zeros1[:], dummy512[:], start=True, stop=True)


    bb_pitch = bb[:].ap[0][0]
    bb_off = bb[:].offset
    fb_pitch = fb[:].ap[0][0]
    fb_off = fb[:].offset
    par = [0]      # parity counter for DVE/ACT copy balancing
    bankctr = [0]  # m3 scores-bank alternation
    ckctr = [0]    # global chunk counter

    # deferred-emission pipeline queues. Each m2q/m3q entry is (mm, post):
    # the mm parts of a pair/bank are emitted NEWEST-FIRST so the first
    # matmul's semaphore wait subsumes the older ones — the rest arrive at
    # the PE wait-free and overlap via their disjoint tile_position groups
    # (the same reason the final stage's shared-dep matmuls overlap).
    m2q, m3q = [], []
    finq = []
    zsums = {}
    trans_prev = [None]
    final_prev = [None]

    def pump_m3(force=False):
        while m3q:
            key = m3q[0][0]
            nbank = sum(1 for k, _ in m3q if k == key)
            rest = len(m3q) - nbank
            if rest < 6 and not force:
                return
            units = [m3q.pop(0)[1] for _ in range(nbank)]
            for u in reversed(units):
                u[0]()
            for u in units:
                u[1]()

    def sel_mm(ps2, base, meta):
        sb_, s0_, ns_, tsb_ = meta
        cols_ = ns_ * tsb_
        lhs = seltile[96:112, sb_ * H2:sb_ * H2 + H2]
        # identity columns broadcast over tsb via 0-stride: the selector
        # pattern P[j,c] = (c//tsb == j - s0) without any stored pattern
        rhs = bass.AP(tensor=seltile[:].tensor,
                      offset=seltile[:].offset + 96 * sel_pitch
                      + NSB * H2 + s0_,
                      ap=[[sel_pitch, 16], [1, ns_], [0, tsb_]])
        nc.tensor.matmul(ps2[base:base + H2, 0:cols_], lhs, rhs,
                         start=False, stop=True,
                         tile_position=(96, base), skip_group_check=True)

    def pump(force=False):
        while len(m2q) >= (2 if force else 3):
            ua, ub = m2q.pop(0), m2q.pop(0)
            ps2 = ps2_pool.tile([128, 512], F32, tag="ps2",
                                name=f"ps2p_{ckctr[0]}_{len(m2q)}")
            ub[0](ps2, 64)
            ua[0](ps2, 0)
            sel_mm(ps2, 0, ua[3])
            sel_mm(ps2, 64, ub[3])
            maxc = max(ua[2], ub[2])
            _mark(nc, 'relu2')
            h2 = h2_pool.tile([64 + H2, 512], BF16, tag="h2",
                              name=f"h2p_{ckctr[0]}_{len(m2q)}")
            nc.scalar.activation(h2[:, 0:maxc], ps2[0:64 + H2, 0:maxc],
                                 AF.Relu, scale=1.0)
            ua[1](h2, 0)
            ub[1](h2, 64)
        if force and m2q:
            ps2 = ps2_pool.tile([128, 512], F32, tag="ps2", name="ps2last")
            u = m2q.pop(0)
            u[0](ps2, 0)
            sel_mm(ps2, 0, u[3])
            _mark(nc, 'relu2')
            h2 = h2_pool.tile([64 + H2, 512], BF16, tag="h2", name="h2last")
            nc.scalar.activation(h2[0:H2, 0:u[2]], ps2[0:H2, 0:u[2]],
                                 AF.Relu, scale=1.0)
            u[1](h2, 0)
        pump_m3(force)

    for gi, g in enumerate(GORDER):
        tg = tgs[g]
        scores = grp_pool.tile([GSLOTS, tg], F32, tag="scores", name=f"scores_g{g}")
        _mark(nc, 'memset')
        nc.gpsimd.memset(scores[:], 0.0)
        emit_ktg(g)
        ktg = ktgs.pop(g)
        kt0 = kt_offs[g * GROUP_SBS]
        knats = {}
        for isb in range(GROUP_SBS):
            sb = g * GROUP_SBS + isb
            tsb, nch, ns = t_sbs[sb], nchs[sb], ns_sbs[sb]
            nchunks = SB // ns
            kto = kt_offs[sb] - kt0
            wss = wss_pre.pop(sb)
            # prefetch the fused weights two sub-blocks ahead so the m1
            # stream never waits on the ws DMA
            pos_ = SB_ORDER.index(sb)
            for sbn in SB_ORDER[pos_ + 1:pos_ + 3]:
                if sbn not in wss_pre:
                    _mark(nc, 'dma_ws')
                    t_ = ws_pool.tile([H, SB * H1], BF16, tag="wss",
                                      name=f"wss_{sbn}")
                    nc.sync.dma_start(t_[:],
                                      ws_d[:, sbn * SB * H1:(sbn + 1) * SB * H1])
                    wss_pre[sbn] = t_
            _mark(nc, 'dma_kn')
            kn = kn_pool.tile([128, SB * nch * 128], BF16, tag="kn", name=f"kn_{sb}")
            o_ = kn_offs[sb]
            for c_, cl_ in enumerate(kn_cls[sb]):
                src = bass.AP(tensor=knat_d.tensor, offset=o_,
                              ap=[[SB * H, cl_], [1, SB * H]])
                nc.sync.dma_start(kn[0:cl_, c_ * SB * H:(c_ + 1) * SB * H], src)
                o_ += cl_ * SB * H
            knats[sb] = (kn, nch)

            for bank in range(-(-nchunks // 4)):
                cis = range(4 * bank, min(nchunks, 4 * bank + 4))
                cols = ns * tsb
                for ci in cis:
                    s0 = ci * ns
                    coff = kto + s0 * tsb
                    ckctr[0] += 1
                    _mark(nc, 'm1')
                    ps1 = ps1_pool.tile([H1, cols], F32, tag="ps1",
                                        name=f"ps1_{sb}_{ci}")
                    for j in range(ns):
                        si = (s0 + j) * H1
                        nc.tensor.matmul(ps1[:, j * tsb:(j + 1) * tsb],
                                         wss[:, si:si + H1],
                                         ktg[:, coff + j * tsb:coff + (j + 1) * tsb],
                                         start=(j == 0), stop=(j == ns - 1),
                                         skip_group_check=True)
                    _mark(nc, 'relu1')
                    h1 = h1_pool.tile([H1, cols], BF16, tag="h1",
                                      name=f"h1_{sb}_{ci}")
                    nb1v = bass.AP(tensor=fb[:].tensor,
                                   offset=fb_off + 2 + NGROUPS + sb * SB + s0,
                                   ap=[[fb_pitch, H1], [1, ns], [0, tsb]])
                    nc.vector.tensor_tensor(h1[:], ps1[:], nb1v, op=A.max)

                    def m2_mm(ps2, base, cols=cols, h1=h1):
                        _mark(nc, 'm2')
                        nc.tensor.matmul(ps2[base:base + H2, 0:cols], w2,
                                         h1[:], start=True, stop=False,
                                         tile_position=(0, base),
                                         skip_group_check=True)

                    def m2_post(h2, rowbase, sb=sb, isb=isb, bank=bank, ci=ci,
                                cols=cols, tsb=tsb, ns=ns,
                                last=(ci == cis[-1]), npg=len(cis),
                                scores=scores):
                        def m3_mm():
                            _mark(nc, 'm3')
                            pg = ci % 4
                            pss = pss_t[bankctr[0] % 2]
                            nc.tensor.matmul(pss[32 * pg:32 * pg + 1, 0:cols],
                                             bb[rowbase:rowbase + H2,
                                                SLOTS + H1:SLOTS + H1 + 1],
                                             h2[rowbase:rowbase + H2, 0:cols],
                                             start=True, stop=True,
                                             tile_position=(rowbase, 32 * pg),
                                             skip_group_check=True)

                        def m3_post():
                            if not last:
                                return
                            pss = pss_t[bankctr[0] % 2]
                            bankctr[0] += 1
                            _mark(nc, 'scr_copy')
                            scratch = scr_pool.tile([128, 512], F32, tag="scr",
                                                    name=f"scr_{sb}_{bank}")
                            nc.scalar.activation(scratch[:, 0:cols],
                                                 pss[:, 0:cols], AF.Exp,
                                                 bias=0.0, scale=SCALE)
                            _mark(nc, 'redis_s')
                            scr_pitch = scratch[:].ap[0][0]
                            src = bass.AP(tensor=scratch[:].tensor,
                                          offset=scratch[:].offset,
                                          ap=[[32 * scr_pitch, npg],
                                              [tsb, ns], [1, tsb]])
                            r0 = 16 * isb + 4 * bank * ns
                            nc.gpsimd.dma_start(
                                scores[r0:r0 + npg * ns, 0:tsb], src)

                        m3q.append(((sb, bank), (m3_mm, m3_post)))

                    m2q.append((m2_mm, m2_post, cols,
                                (sb, s0, ns, tsb)))
                    pump()
            if isb == 0 and finq:
                finq.pop(0)()
            if isb == 1:
                if finq:
                    finq.pop(0)()
                if gi + 1 < NGROUPS:
                    emit_ktg(GORDER[gi + 1])
            if isb == 2:
                if trans_prev[0] is not None:
                    trans_prev[0]()
                    trans_prev[0] = None
                if final_prev[0] is not None:
                    finq.extend(final_prev[0]())
                    final_prev[0] = None
                for _ in range(2):
                    if finq:
                        finq.pop(0)()
            if isb == 3:
                for _ in range(2):
                    if finq:
                        finq.pop(0)()
        pump(force=True)

        atts = []

        def trans_unit(g=g, tg=tg, scores=scores, atts=atts):
            # softmax mostly on the otherwise-idle gpsimd so DVE/ACT queues
            # stay clear for the relu stream
            _mark(nc, 'softmax')
            pm = grp_pool.tile([GSLOTS, tg], F32, tag="pm", name=f"pm_{g}")
            nc.vector.tensor_tensor(pm[:], scores[:], masks[g][:], op=A.mult)
            zsum = grp_pool.tile([GSLOTS, 1], F32, tag="zsum", name=f"zsum_{g}")
            nc.vector.reduce_sum(zsum[:], pm[:], axis=mybir.AxisListType.X)
            rz = grp_pool.tile([GSLOTS, 1], F32, tag="rz", name=f"rz_{g}")
            nc.vector.reciprocal(rz[:], zsum[:])
            attnb = grp_pool.tile([GSLOTS, tg], BF16, tag="attnb",
                                  name=f"attnb_{g}")
            nc.vector.tensor_scalar_mul(attnb[:], pm[:], rz[:, 0:1])
            _mark(nc, 'transpose')
            for c in range(-(-tg // 128)):
                cl = min(128, tg - 128 * c)
                ps_t = pst_pool.tile([cl, GSLOTS], BF16, tag="pst",
                                     name=f"pst_{g}_{c}")
                nc.tensor.transpose(ps_t[:], attnb[:, 128 * c:128 * c + cl],
                                    identb[0:GSLOTS, 0:GSLOTS])
                at = at_pool.tile([cl, GSLOTS], BF16, tag="at", name=f"at_{g}_{c}")
                nc.scalar.copy(at[:], ps_t[:])
                atts.append(at)

        def make_final_units(g=g, knats=knats, atts=atts):
            oscr_box = [None]

            def f_sb(isb):
                def run():
                    _mark(nc, 'final')
                    if oscr_box[0] is None:
                        oscr_box[0] = osc_pool.tile(
                            [128, GROUP_SBS * 512], F32, tag="oscr",
                            name=f"oscr_{g}")
                    oscr = oscr_box[0]
                    sb = g * GROUP_SBS + isb
                    tsb = t_sbs[sb]
                    kn, nch = knats[sb]
                    ps_o = pso_t[sb % 2]
                    if nch == 1:
                        for ssb in range(SB):
                            r = 16 * isb + ssb
                            cl = tsb
                            blk = ssb * 128
                            nc.tensor.matmul(
                                ps_o[32 * (ssb % 4):32 * (ssb % 4) + 1,
                                     128 * (ssb // 4):128 * (ssb // 4) + 128],
                                atts[0][0:cl, r:r + 1], kn[0:cl, blk:blk + 128],
                                start=True, stop=True,
                                tile_position=(0, 32 * (ssb % 4)),
                                skip_group_check=True)
                    else:
                        nc.tensor.matmul(ps_o[:], zeros1[:], dummy512[:],
                                         start=True, stop=False,
                                         skip_group_check=True)
                        for c in range(nch):
                            cl = min(128, tsb - 128 * c)
                            for ssb in range(SB):
                                r = 16 * isb + ssb
                                blk = c * SB * H + ssb * 128
                                nc.tensor.matmul(
                                    ps_o[32 * (ssb % 4):32 * (ssb % 4) + 1,
                                         128 * (ssb // 4):128 * (ssb // 4) + 128],
                                    atts[c][0:cl, r:r + 1],
                                    kn[0:cl, blk:blk + 128],
                                    start=False, stop=(c == nch - 1),
                                    tile_position=(0, 32 * (ssb % 4)),
                                    skip_group_check=True)
                    _mark(nc, 'oscr_copy')
                    nc.scalar.copy(oscr[:, isb * 512:isb * 512 + 512], ps_o[:])
                return run

            def f_out():
                _mark(nc, 'out_dma')
                oscr = oscr_box[0]
                os_pitch = oscr[:].ap[0][0]
                src = bass.AP(tensor=oscr[:].tensor, offset=oscr[:].offset,
                              ap=[[32 * os_pitch, 4], [512, 4], [128, 4],
                                  [1, 128]])
                nc.sync.dma_start(out_d[g * GSLOTS:(g + 1) * GSLOTS, :], src)

            return [f_sb(0), f_sb(1), f_sb(2), f_sb(3), f_out]

        trans_prev[0] = trans_unit
        final_prev[0] = make_final_units
    trans_prev[0]()
    for u in final_prev[0]():
        u()
    while finq:
        finq.pop(0)()


def pack_inputs(query, keys, keys_length, W1, b1, W2, b2, Wf, bf, plan):
    """Build the 8 per-core input maps."""
    bmap, t_sbs, nchs = plan["bmap"], plan["t_sbs"], plan["nchs"]
    kt_w, kn_w = plan["kt_w"], plan["kn_w"]
    Wq = (W1[0:H] + W1[2 * H:3 * H]).astype(np.float32)
    Wk = (W1[H:2 * H] - W1[2 * H:3 * H]).astype(np.float32)
    Wqk = W1[3 * H:4 * H].astype(np.float32)
    bfl = ml_dtypes.bfloat16
    in_maps = []
    for c in range(NC):
        ktp = np.zeros((H, kt_w), bfl)
        knp = np.zeros((kn_w,), bfl)
        qtp = np.zeros((H, SLOTS), np.float32)
        lensp = np.zeros((GSLOTS, NGROUPS), np.float32)
        for sb in range(NSB):
            tsb, nch = t_sbs[sb], nchs[sb]
            ko, no = plan["kt_offs"][sb], plan["kn_offs"][sb]
            g, isb = sb // GROUP_SBS, sb % GROUP_SBS
            for ssb in range(SB):
                s = sb * SB + ssb
                b = int(bmap[s, c])
                ktp[:, ko + ssb * tsb: ko + (ssb + 1) * tsb] = keys[b, :tsb, :].T
                o_ = no
                for ch, cl in enumerate(plan["kn_cls"][sb]):
                    view = knp[o_:o_ + cl * SB * H].reshape(cl, SB, H)
                    view[:, ssb, :] = keys[b, 128 * ch:128 * ch + cl, :]
                    o_ += cl * SB * H
                qtp[:, s] = query[b]
                lensp[16 * isb + ssb, g] = keys_length[b]
        # per-slot fused first-layer weights: Ws = Wk + diag(q_s) Wqk
        wsp = (Wk[:, None, :] + qtp[:, :, None] * Wqk[:, None, :])  # [H,SLOTS,H1]
        wsp = wsp.reshape(H, SLOTS * H1).astype(bfl)
        # bf16 const bundle: [qt | wq | wf(x2 bands) | w2]
        bbp = np.zeros((H, SLOTS + H1 + 1 + H2), bfl)
        bbp[:, 0:SLOTS] = qtp
        bbp[:, SLOTS:SLOTS + H1] = Wq
        bbp[0:H2, SLOTS + H1] = Wf[:, 0]
        bbp[64:64 + H2, SLOTS + H1] = Wf[:, 0]
        bbp[0:H1, SLOTS + H1 + 1:] = W2
        # f32 const bundle: [b1 | b2(x2 bands) | lens | -b1eff per slot]
        b1e = (Wq.T.astype(np.float32) @ qtp) + b1[:, None]      # [H1, SLOTS]
        fbp = np.zeros((128, 2 + NGROUPS + SLOTS), np.float32)
        fbp[0:H1, 0] = b1
        fbp[0:H2, 1] = b2
        fbp[64:64 + H2, 1] = b2
        fbp[0:GSLOTS, 2:2 + NGROUPS] = lensp
        fbp[0:H1, 2 + NGROUPS:] = -b1e
        # sel table at partitions 96-111: b2effT per sub-block + chunk patterns
        b2e = b2[:, None] + W2.astype(np.float32).T @ b1e        # [H2, SLOTS]
        selp = np.zeros((16, plan["sel_w"]), bfl)
        for sb in range(NSB):
            for j in range(SB):
                selp[j, sb * H2:(sb + 1) * H2] = b2e[:, sb * SB + j]
        for j in range(SB):
            selp[j, NSB * H2 + j] = 1.0
        in_maps.append({"keysT": ktp, "knat": knp, "bb": bbp, "fb": fbp,
                        "ws": wsp, "sel": selp})
    return in_maps


def build_program(plan):
    nc = bacc.Bacc("TRN2", num_devices=NC)
    ins = [
        nc.dram_tensor("keysT", [H, plan["kt_w"]], BF16, kind="ExternalInput").ap(),
        nc.dram_tensor("knat", [plan["kn_w"]], BF16, kind="ExternalInput").ap(),
        nc.dram_tensor("bb", [H, SLOTS + H1 + 1 + H2], BF16,
                       kind="ExternalInput").ap(),
        nc.dram_tensor("fb", [128, 2 + NGROUPS + SLOTS], F32,
                       kind="ExternalInput").ap(),
        nc.dram_tensor("ws", [H, SLOTS * H1], BF16, kind="ExternalInput").ap(),
        nc.dram_tensor("sel", [16, plan["sel_w"]], BF16,
                       kind="ExternalInput").ap(),
        plan["sel_w"],
    ]
    outs = [nc.dram_tensor("outN", [SLOTS, H], F32, kind="ExternalOutput").ap()]
    with tile.TileContext(nc) as tc:
        with ExitStack() as ctx:
            build_body(ctx, tc, outs, ins, plan)
    nc.compile()
    return nc


last_results = None  # stash for external profiling/analysis


def kernel(query, keys, keys_length, W1, b1, W2, b2, Wf, bf):
    global last_results
    from concourse.bass_utils import run_bass_kernel_spmd
    query = np.asarray(query, np.float32)
    keys = np.asarray(keys, np.float32)
    keys_length = np.asarray(keys_length)
    plan = make_plan(keys_length)
    in_maps = pack_inputs(query, keys, keys_length, np.asarray(W1, np.float32),
                          np.asarray(b1, np.float32), np.asarray(W2, np.float32),
                          np.asarray(b2, np.float32), np.asarray(Wf, np.float32),
                          np.asarray(bf, np.float32), plan)
    nc = build_program(plan)
    trace = bool(int(os.environ.get("BASS_KERNEL_TRACE", "0")))
    res = run_bass_kernel_spmd(nc, in_maps, core_ids=list(range(NC)), trace=trace)
    last_results = res
    globals()["last_nc"] = nc
    if trace and res.exec_time_ns is not None:
        print(f"HW exec time: {res.exec_time_ns} ns")
    out = np.zeros((B, H), np.float32)
    bmap = plan["bmap"]
    rows = np.array([GSLOTS * (s // GSLOTS)
                     + _out_row_in_group((s // SB) % GROUP_SBS, s % SB)
                     for s in range(SLOTS)])
    for c in range(NC):
        outN = res.results[c]["outN"]  # [SLOTS, H]
        out[bmap[:, c]] = outN[rows]
    return out



# revision 46
# speedup vs baseline: 1.0969x; 1.0763x over previous
"""DIN-attention Trainium2 kernel.

out[b] = softmax_t(MLP(concat[q, k, q-k, q*k]) / sqrt(H), mask=t<len_b) @ keys[b]

Strategy (8-core data parallel over B, one shared SPMD program):
- Host sorts b by keys_length, deals round-robin to cores -> per-core slot s
  holds similar lengths on every core; per 16-slot sub-block, work is
  truncated to the sub-block max length (halves all work in expectation).
- MLP decomposition: din@W1 = k@(Wk + diag(q)Wqk) + (Wq^T q + b1) with
  Wk=W1b-W1c, Wqk=W1d, Wq=W1a+W1c; the per-slot fused weight
  Ws = Wk + diag(q_s)Wqk is packed on the host, so m1 is one matmul per
  slot; the per-slot bias (Wq^T q_s + b1) enters the PSUM accumulation
  group through a small K=ns+1 selector matmul pinned to PE row-group 3
  (tile_position=(96,0)), which runs concurrently with the m2 pair
  (K=80, row-groups 0-2) on disjoint PE sub-arrays.
- m2 chunks are emitted in pairs packed at col-groups (0,0)/(0,64) for
  2x PE concurrency; m3 is a single M=1 matmul per chunk packed at PSUM
  partition groups {0,32,64,96} via tile_position, bank-batched.
- Deferred-emission software pipeline (bias/relu1 -> m2 pair/relu2 -> m3
  banks) keeps every matmul's dependencies satisfied before it reaches the
  PE queue head; group g's transpose + output contraction are emitted
  inside group g+1's first sub-block.
- Output contraction runs attn columns (PE-transposed, bf16) as M=1
  stationaries against native-layout bf16 keys with tile_position cycling
  (4x concurrency), accumulated across 128-row chunks over a pre-zeroed
  PSUM bank.
"""

import os
import sys
from contextlib import ExitStack

for _p in ("/opt/trn_rl_repo",):
    if _p not in sys.path:
        sys.path.insert(0, _p)

os.environ.setdefault("CONCOURSE_ENABLE_LDW_OPT", "false")

import numpy as np
import ml_dtypes

import concourse.bass as bass
import concourse.tile as tile
from concourse import bacc, mybir
from concourse.masks import make_identity

F32 = mybir.dt.float32
BF16 = mybir.dt.bfloat16
A = mybir.AluOpType
AF = mybir.ActivationFunctionType

B, T, H = 2048, 200, 128
H1, H2 = 80, 40
NC = 8
SLOTS = B // NC          # 256 slots per core
SB = 16                  # slots per sub-block
NSB = SLOTS // SB        # 16 sub-blocks per core
GROUP_SBS = 4            # sub-blocks per softmax group
NGROUPS = NSB // GROUP_SBS
GSLOTS = GROUP_SBS * SB  # 64 slots per group
SCALE = float(1.0 / np.sqrt(np.float32(H)))
NEG = -1e9


def _roundup(x, m):
    return ((int(x) + m - 1) // m) * m


def _ns_of(tsb):
    # uniform power-of-two slots per chunk with ns*tsb <= 512
    for ns in (16, 8, 4, 2):
        if ns * tsb <= 512:
            return ns
    return 1


def make_plan(keys_length):
    """Global plan shared by all cores: slot assignment + per-sub-block T."""
    order = np.argsort(keys_length, kind="stable")
    bmap = order.reshape(SLOTS, NC)          # [slot, core] -> b
    t_sbs = []
    for sb in range(NSB):
        lens = np.asarray(keys_length)[bmap[sb * SB:(sb + 1) * SB]]
        m = int(lens.max())
        t_sbs.append(min(T, _roundup(m, 8)))
    nchs = [max(1, -(-t // 128)) for t in t_sbs]
    ns_sbs = [_ns_of(t) for t in t_sbs]
    kt_offs, off = [], 0
    for t in t_sbs:
        kt_offs.append(off)
        off += SB * t
    kt_w = off
    # kn: flat, per sub-block, grouped by 128-row chunk; only valid key rows
    # are stored/transferred (cl rows per chunk) to cut HBM traffic.
    kn_cls = [[min(128, t - 128 * c) for c in range(nchs[i])]
              for i, t in enumerate(t_sbs)]
    kn_offs, off = [], 0
    for sb in range(NSB):
        kn_offs.append(off)
        off += sum(kn_cls[sb]) * SB * H
    kn_w = off
    tgs = [max(t_sbs[g * GROUP_SBS:(g + 1) * GROUP_SBS]) for g in range(NGROUPS)]
    # canonical chunk order (must match build_body's emission loops)
    chunks = []
    for sb in range(NSB):
        ns = ns_sbs[sb]
        for ci in range(SB // ns):
            chunks.append((sb, ci, ci * ns, ns, t_sbs[sb]))
    tsb_vals = sorted(set(t_sbs))
    return dict(bmap=bmap, t_sbs=t_sbs, nchs=nchs, ns_sbs=ns_sbs,
                kt_offs=kt_offs, kt_w=kt_w, kn_offs=kn_offs, kn_w=kn_w,
                kn_cls=kn_cls, tgs=tgs, chunks=chunks, tsb_vals=tsb_vals)


def _out_row_in_group(isb, ssb):
    # row of slot ssb (of sub-block isb) within the group's packed out DMA:
    # traversal order (pgroup, isb, colblk)
    return (ssb % 4) * 16 + isb * 4 + ssb // 4


SECTION_MARKS = []


def _mark(nc, label):
    SECTION_MARKS.append((len(nc.inst_map), label))


def build_body(ctx, tc, outs, ins, plan):
    nc = tc.nc
    SECTION_MARKS.clear()
    keysT_d, knat_d, bb_d, fb_d, ws_d = ins
    out_d, = outs
    t_sbs, nchs, ns_sbs = plan["t_sbs"], plan["nchs"], plan["ns_sbs"]
    kt_offs, kn_offs, tgs = plan["kt_offs"], plan["kn_offs"], plan["tgs"]
    kn_cls = plan["kn_cls"]

    singles = ctx.enter_context(tc.tile_pool(name="singles", bufs=1))
    kt_pool = ctx.enter_context(tc.tile_pool(name="kt", bufs=2))
    kn_pool = ctx.enter_context(tc.tile_pool(name="kn", bufs=2 * GROUP_SBS))
    ws_pool = ctx.enter_context(tc.tile_pool(name="ws", bufs=3))
    h1_pool = ctx.enter_context(tc.tile_pool(name="h1", bufs=8))
    h2_pool = ctx.enter_context(tc.tile_pool(name="h2", bufs=8))
    scr_pool = ctx.enter_context(tc.tile_pool(name="scr", bufs=6))
    osc_pool = ctx.enter_context(tc.tile_pool(name="osc", bufs=3))
    grp_pool = ctx.enter_context(tc.tile_pool(name="grp", bufs=2))
    at_pool = ctx.enter_context(tc.tile_pool(name="at", bufs=4))
    ps1_pool = ctx.enter_context(tc.tile_pool(name="ps1", bufs=2, space="PSUM"))
    pst_pool = ctx.enter_context(tc.tile_pool(name="pst", bufs=1, space="PSUM"))
    ps2_pool = ctx.enter_context(tc.tile_pool(name="ps2", bufs=2, space="PSUM"))
    psper_pool = ctx.enter_context(tc.tile_pool(name="psper", bufs=1, space="PSUM"))

    # ---- first kt group + first sub-block weights: prefetch before the
    # constants so the opening m1 isn't starved behind small DMAs ----
    ktgs = {}
    kt_pool_ref = kt_pool

    def emit_ktg(g, split=False):
        if g >= NGROUPS or g in ktgs:
            return
        _mark(nc, 'dma_kt')
        kt0_ = kt_offs[g * GROUP_SBS]
        ktg_w = sum(SB * t_sbs[g * GROUP_SBS + i] for i in range(GROUP_SBS))
        t = kt_pool_ref.tile([H, ktg_w], BF16, tag="kt", name=f"ktg_{g}")
        if split:
            # three transfers: sb0 | sb1 | sb2+sb3, so the opening m1 can
            # begin as soon as the first (small) slice has landed
            w0 = SB * t_sbs[g * GROUP_SBS]
            w1 = w0 + SB * t_sbs[g * GROUP_SBS + 1]
            nc.sync.dma_start(t[:, 0:w0], keysT_d[:, kt0_:kt0_ + w0])
            nc.sync.dma_start(t[:, w0:w1], keysT_d[:, kt0_ + w0:kt0_ + w1])
            nc.sync.dma_start(t[:, w1:ktg_w],
                              keysT_d[:, kt0_ + w1:kt0_ + ktg_w])
        else:
            nc.sync.dma_start(t[:], keysT_d[:, kt0_:kt0_ + ktg_w])
        ktgs[g] = t

    GORDER = [1, 2, 3, 0]
    SB_ORDER = [g_ * GROUP_SBS + i_ for g_ in GORDER for i_ in range(GROUP_SBS)]
    # startup order matters: each DMA trigger occupies the queue ~0.6-0.9us,
    # so the first m1's dependencies (kt sb0 slice, const bundles, wss0) are
    # triggered FIRST, in 4 transfers total.
    sbf = SB_ORDER[0]
    kt0_ = kt_offs[sbf]
    w0_ = SB * t_sbs[sbf]
    ktg_w0 = sum(SB * t_sbs[sbf + i] for i in range(GROUP_SBS))
    ktg0 = kt_pool.tile([H, ktg_w0], BF16, tag="kt", name=f"ktg_{GORDER[0]}")
    nc.sync.dma_start(ktg0[:, 0:w0_], keysT_d[:, kt0_:kt0_ + w0_])
    wss_first = ws_pool.tile([H, SB * H1], BF16, tag="wss", name=f"wss_{sbf}")
    nc.sync.dma_start(wss_first[:], ws_d[:, sbf * SB * H1:(sbf + 1) * SB * H1])
    # bb: [qt | wq | wf2 | w2] bf16 bundle, fb: [b1 | b2(x2) | lens] f32 bundle
    bb = singles.tile([H, SLOTS + H1 + 1 + H2], BF16, name="bb")
    nc.sync.dma_start(bb[:], bb_d)
    fb = singles.tile([128, 2 + NGROUPS], F32, name="fb")
    nc.sync.dma_start(fb[:], fb_d)
    qt = bb[:, 0:SLOTS]
    wq = bb[:, SLOTS:SLOTS + H1]
    wf2 = bb[:, SLOTS + H1:SLOTS + H1 + 1]
    w2 = bb[0:H1, SLOTS + H1 + 1:SLOTS + H1 + 1 + H2]
    b1c = fb[0:H1, 0:1]
    b2c = fb[0:H2, 1:2]
    b2c2 = fb[0:64 + H2, 1:2]
    lens = fb[0:GSLOTS, 2:2 + NGROUPS]
    wss_pre = {sbf: wss_first}
    t_ = ws_pool.tile([H, SB * H1], BF16, tag="wss", name=f"wss_{sbf + 1}")
    nc.sync.dma_start(t_[:], ws_d[:, (sbf + 1) * SB * H1:(sbf + 2) * SB * H1])
    wss_pre[sbf + 1] = t_
    # rest of the first group's keysT
    w1_ = w0_ + SB * t_sbs[sbf + 1]
    nc.sync.dma_start(ktg0[:, w0_:w1_], keysT_d[:, kt0_ + w0_:kt0_ + w1_])
    nc.sync.dma_start(ktg0[:, w1_:ktg_w0], keysT_d[:, kt0_ + w1_:kt0_ + ktg_w0])
    ktgs[GORDER[0]] = ktg0
    iota = singles.tile([128, T], F32, name="iota")
    nc.gpsimd.iota(iota[:], pattern=[[1, T]], base=0, channel_multiplier=0,
                   allow_small_or_imprecise_dtypes=True)
    identb = singles.tile([128, 128], BF16, name="identb")
    make_identity(nc, identb[:])
    zeros1 = singles.tile([1, 128], BF16, name="zeros1")
    nc.vector.memset(zeros1[:], 0.0)
    dummy512 = singles.tile([1, 512], BF16, name="dummy512")
    nc.vector.memset(dummy512[:], 0.0)
    # per-group per-slot length masks (t < len), on the otherwise-idle gpsimd
    masks = []
    for g in range(NGROUPS):
        tg = tgs[g]
        mk = singles.tile([GSLOTS, tg], F32, name=f"mask_{g}")
        nc.gpsimd.tensor_scalar(mk[:], iota[0:GSLOTS, 0:tg],
                                fb[0:GSLOTS, 2 + g:3 + g],
                                None, op0=A.is_lt)
        masks.append(mk)
    # persistent, one-time-zeroed PSUM banks
    pss_t = [psper_pool.tile([128, 512], F32, tag=f"pssp{i}", name=f"pssp{i}")
             for i in range(2)]
    pso_t = [psper_pool.tile([128, 512], F32, tag="psop0", name="psop0")] * 2
    for t_ in pss_t + pso_t[:1]:
        nc.tensor.matmul(t_[:], zeros1[:], dummy512[:], start=True, stop=True)


    bb_pitch = bb[:].ap[0][0]
    bb_off = bb[:].offset
    par = [0]      # parity counter for DVE/ACT copy balancing
    bankctr = [0]  # m3 scores-bank alternation
    ckctr = [0]    # global chunk counter

    # deferred-emission pipeline queues. Each m2q/m3q entry is (mm, post):
    # the mm parts of a pair/bank are emitted NEWEST-FIRST so the first
    # matmul's semaphore wait subsumes the older ones — the rest arrive at
    # the PE wait-free and overlap via their disjoint tile_position groups
    # (the same reason the final stage's shared-dep matmuls overlap).
    m2q, m3q = [], []
    finq = []
    zsums = {}
    trans_prev = [None]
    final_prev = [None]

    def pump_m3(force=False):
        while m3q:
            key = m3q[0][0]
            nbank = sum(1 for k, _ in m3q if k == key)
            rest = len(m3q) - nbank
            if rest < 6 and not force:
                return
            units = [m3q.pop(0)[1] for _ in range(nbank)]
            for u in reversed(units):
                u[0]()
            for u in units:
                u[1]()

    def pump(force=False):
        while len(m2q) >= (2 if force else 3):
            ua, ub = m2q.pop(0), m2q.pop(0)
            ps2 = ps2_pool.tile([128, 512], F32, tag="ps2",
                                name=f"ps2p_{ckctr[0]}_{len(m2q)}")
            ub[0](ps2, 64)
            ua[0](ps2, 0)
            maxc = max(ua[2], ub[2])
            _mark(nc, 'relu2')
            h2 = h2_pool.tile([64 + H2, 512], BF16, tag="h2",
                              name=f"h2p_{ckctr[0]}_{len(m2q)}")
            nc.scalar.activation(h2[:, 0:maxc], ps2[0:64 + H2, 0:maxc],
                                 AF.Relu, bias=b2c2, scale=1.0)
            ua[1](h2, 0)
            ub[1](h2, 64)
        if force and m2q:
            ps2 = ps2_pool.tile([128, 512], F32, tag="ps2", name="ps2last")
            u = m2q.pop(0)
            u[0](ps2, 0)
            _mark(nc, 'relu2')
            h2 = h2_pool.tile([64 + H2, 512], BF16, tag="h2", name="h2last")
            nc.scalar.activation(h2[0:H2, 0:u[2]], ps2[0:H2, 0:u[2]],
                                 AF.Relu, bias=b2c, scale=1.0)
            u[1](h2, 0)
        pump_m3(force)

    for gi, g in enumerate(GORDER):
        tg = tgs[g]
        scores = grp_pool.tile([GSLOTS, tg], F32, tag="scores", name=f"scores_g{g}")
        _mark(nc, 'memset')
        nc.gpsimd.memset(scores[:], 0.0)
        emit_ktg(g)
        ktg = ktgs.pop(g)
        kt0 = kt_offs[g * GROUP_SBS]
        knats = {}
        for isb in range(GROUP_SBS):
            sb = g * GROUP_SBS + isb
            tsb, nch, ns = t_sbs[sb], nchs[sb], ns_sbs[sb]
            nchunks = SB // ns
            kto = kt_offs[sb] - kt0
            wss = wss_pre.pop(sb)
            # prefetch the fused weights two sub-blocks ahead so the m1
            # stream never waits on the ws DMA
            pos_ = SB_ORDER.index(sb)
            for sbn in SB_ORDER[pos_ + 1:pos_ + 3]:
                if sbn not in wss_pre:
                    _mark(nc, 'dma_ws')
                    t_ = ws_pool.tile([H, SB * H1], BF16, tag="wss",
                                      name=f"wss_{sbn}")
                    nc.sync.dma_start(t_[:],
                                      ws_d[:, sbn * SB * H1:(sbn + 1) * SB * H1])
                    wss_pre[sbn] = t_
            _mark(nc, 'dma_kn')
            kn = kn_pool.tile([128, SB * nch * 128], BF16, tag="kn", name=f"kn_{sb}")
            o_ = kn_offs[sb]
            for c_, cl_ in enumerate(kn_cls[sb]):
                src = bass.AP(tensor=knat_d.tensor, offset=o_,
                              ap=[[SB * H, cl_], [1, SB * H]])
                nc.sync.dma_start(kn[0:cl_, c_ * SB * H:(c_ + 1) * SB * H], src)
                o_ += cl_ * SB * H
            knats[sb] = (kn, nch)

            for bank in range(-(-nchunks // 4)):
                cis = range(4 * bank, min(nchunks, 4 * bank + 4))
                cols = ns * tsb
                for ci in cis:
                    s0 = ci * ns
                    coff = kto + s0 * tsb
                    ckctr[0] += 1
                    _mark(nc, 'm1')
                    ps1 = ps1_pool.tile([H1, cols], F32, tag="ps1",
                                        name=f"ps1_{sb}_{ci}")
                    for j in range(ns):
                        si = (s0 + j) * H1
                        nc.tensor.matmul(ps1[:, j * tsb:(j + 1) * tsb],
                                         wss[:, si:si + H1],
                                         ktg[:, coff + j * tsb:coff + (j + 1) * tsb],
                                         start=(j == 0), stop=False,
                                         skip_group_check=True)
                    qview = bass.AP(tensor=bb[:].tensor,
                                    offset=bb_off + sb * SB + s0,
                                    ap=[[bb_pitch, H], [1, ns], [0, tsb]])
                    nc.tensor.matmul(ps1[:, 0:cols], wq, qview,
                                     start=False, stop=True,
                                     skip_group_check=True)
                    _mark(nc, 'relu1')
                    h1 = h1_pool.tile([H1, cols], BF16, tag="h1",
                                      name=f"h1_{sb}_{ci}")
                    if ckctr[0] % 3 == 0:
                        nc.scalar.activation(h1[:], ps1[:], AF.Relu,
                                             bias=b1c, scale=1.0)
                    else:
                        nc.vector.tensor_scalar(h1[:], ps1[:], b1c, 0.0,
                                                op0=A.add, op1=A.max)

                    def m2_mm(ps2, base, cols=cols, h1=h1):
                        _mark(nc, 'm2')
                        nc.tensor.matmul(ps2[base:base + H2, 0:cols], w2,
                                         h1[:], start=True, stop=True,
                                         tile_position=(0, base),
                                         skip_group_check=True)

                    def m2_post(h2, rowbase, sb=sb, isb=isb, bank=bank, ci=ci,
                                cols=cols, tsb=tsb, ns=ns,
                                last=(ci == cis[-1]), npg=len(cis),
                                scores=scores):
                        def m3_mm():
                            _mark(nc, 'm3')
                            pg = ci % 4
                            pss = pss_t[bankctr[0] % 2]
                            nc.tensor.matmul(pss[32 * pg:32 * pg + 1, 0:cols],
                                             bb[rowbase:rowbase + H2,
                                                SLOTS + H1:SLOTS + H1 + 1],
                                             h2[rowbase:rowbase + H2, 0:cols],
                                             start=True, stop=True,
                                             tile_position=(rowbase, 32 * pg),
                                             skip_group_check=True)

                        def m3_post():
                            if not last:
                                return
                            pss = pss_t[bankctr[0] % 2]
                            bankctr[0] += 1
                            _mark(nc, 'scr_copy')
                            scratch = scr_pool.tile([128, 512], F32, tag="scr",
                                                    name=f"scr_{sb}_{bank}")
                            nc.scalar.activation(scratch[:, 0:cols],
                                                 pss[:, 0:cols], AF.Exp,
                                                 bias=0.0, scale=SCALE)
                            _mark(nc, 'redis_s')
                            scr_pitch = scratch[:].ap[0][0]
                            src = bass.AP(tensor=scratch[:].tensor,
                                          offset=scratch[:].offset,
                                          ap=[[32 * scr_pitch, npg],
                                              [tsb, ns], [1, tsb]])
                            r0 = 16 * isb + 4 * bank * ns
                            nc.gpsimd.dma_start(
                                scores[r0:r0 + npg * ns, 0:tsb], src)

                        m3q.append(((sb, bank), (m3_mm, m3_post)))

                    m2q.append((m2_mm, m2_post, cols))
                    pump()
            if isb == 0 and finq:
                finq.pop(0)()
            if isb == 1:
                if finq:
                    finq.pop(0)()
                if gi + 1 < NGROUPS:
                    emit_ktg(GORDER[gi + 1])
            if isb == 2:
                if trans_prev[0] is not None:
                    trans_prev[0]()
                    trans_prev[0] = None
                if final_prev[0] is not None:
                    finq.extend(final_prev[0]())
                    final_prev[0] = None
                for _ in range(2):
                    if finq:
                        finq.pop(0)()
            if isb == 3:
                for _ in range(2):
                    if finq:
                        finq.pop(0)()
        pump(force=True)

        atts = []

        def trans_unit(g=g, tg=tg, scores=scores, atts=atts):
            # softmax mostly on the otherwise-idle gpsimd so DVE/ACT queues
            # stay clear for the relu stream
            _mark(nc, 'softmax')
            pm = grp_pool.tile([GSLOTS, tg], F32, tag="pm", name=f"pm_{g}")
            nc.vector.tensor_tensor(pm[:], scores[:], masks[g][:], op=A.mult)
            zsum = grp_pool.tile([GSLOTS, 1], F32, tag="zsum", name=f"zsum_{g}")
            nc.vector.reduce_sum(zsum[:], pm[:], axis=mybir.AxisListType.X)
            rz = grp_pool.tile([GSLOTS, 1], F32, tag="rz", name=f"rz_{g}")
            nc.vector.reciprocal(rz[:], zsum[:])
            attnb = grp_pool.tile([GSLOTS, tg], BF16, tag="attnb",
                                  name=f"attnb_{g}")
            nc.vector.tensor_scalar_mul(attnb[:], pm[:], rz[:, 0:1])
            _mark(nc, 'transpose')
            for c in range(-(-tg // 128)):
                cl = min(128, tg - 128 * c)
                ps_t = pst_pool.tile([cl, GSLOTS], BF16, tag="pst",
                                     name=f"pst_{g}_{c}")
                nc.tensor.transpose(ps_t[:], attnb[:, 128 * c:128 * c + cl],
                                    identb[0:GSLOTS, 0:GSLOTS])
                at = at_pool.tile([cl, GSLOTS], BF16, tag="at", name=f"at_{g}_{c}")
                nc.vector.tensor_copy(at[:], ps_t[:])
                atts.append(at)

        def make_final_units(g=g, knats=knats, atts=atts):
            oscr_box = [None]

            def f_sb(isb):
                def run():
                    _mark(nc, 'final')
                    if oscr_box[0] is None:
                        oscr_box[0] = osc_pool.tile(
                            [128, GROUP_SBS * 512], F32, tag="oscr",
                            name=f"oscr_{g}")
                    oscr = oscr_box[0]
                    sb = g * GROUP_SBS + isb
                    tsb = t_sbs[sb]
                    kn, nch = knats[sb]
                    ps_o = pso_t[sb % 2]
                    if nch == 1:
                        for ssb in range(SB):
                            r = 16 * isb + ssb
                            cl = tsb
                            blk = ssb * 128
                            nc.tensor.matmul(
                                ps_o[32 * (ssb % 4):32 * (ssb % 4) + 1,
                                     128 * (ssb // 4):128 * (ssb // 4) + 128],
                                atts[0][0:cl, r:r + 1], kn[0:cl, blk:blk + 128],
                                start=True, stop=True,
                                tile_position=(0, 32 * (ssb % 4)),
                                skip_group_check=True)
                    else:
                        nc.tensor.matmul(ps_o[:], zeros1[:], dummy512[:],
                                         start=True, stop=False,
                                         skip_group_check=True)
                        for c in range(nch):
                            cl = min(128, tsb - 128 * c)
                            for ssb in range(SB):
                                r = 16 * isb + ssb
                                blk = c * SB * H + ssb * 128
                                nc.tensor.matmul(
                                    ps_o[32 * (ssb % 4):32 * (ssb % 4) + 1,
                                         128 * (ssb // 4):128 * (ssb // 4) + 128],
                                    atts[c][0:cl, r:r + 1],
                                    kn[0:cl, blk:blk + 128],
                                    start=False, stop=(c == nch - 1),
                                    tile_position=(0, 32 * (ssb % 4)),
                                    skip_group_check=True)
                    _mark(nc, 'oscr_copy')
                    nc.vector.tensor_copy(oscr[:, isb * 512:isb * 512 + 512],
                                          ps_o[:])
                return run

            def f_out():
                _mark(nc, 'out_dma')
                oscr = oscr_box[0]
                os_pitch = oscr[:].ap[0][0]
                src = bass.AP(tensor=oscr[:].tensor, offset=oscr[:].offset,
                              ap=[[32 * os_pitch, 4], [512, 4], [128, 4],
                                  [1, 128]])
                nc.sync.dma_start(out_d[g * GSLOTS:(g + 1) * GSLOTS, :], src)

            return [f_sb(0), f_sb(1), f_sb(2), f_sb(3), f_out]

        trans_prev[0] = trans_unit
        final_prev[0] = make_final_units
    trans_prev[0]()
    for u in final_prev[0]():
        u()
    while finq:
        finq.pop(0)()


def pack_inputs(query, keys, keys_length, W1, b1, W2, b2, Wf, bf, plan):
    """Build the 8 per-core input maps."""
    bmap, t_sbs, nchs = plan["bmap"], plan["t_sbs"], plan["nchs"]
    kt_w, kn_w = plan["kt_w"], plan["kn_w"]
    Wq = (W1[0:H] + W1[2 * H:3 * H]).astype(np.float32)
    Wk = (W1[H:2 * H] - W1[2 * H:3 * H]).astype(np.float32)
    Wqk = W1[3 * H:4 * H].astype(np.float32)
    bfl = ml_dtypes.bfloat16
    in_maps = []
    for c in range(NC):
        ktp = np.zeros((H, kt_w), bfl)
        knp = np.zeros((kn_w,), bfl)
        qtp = np.zeros((H, SLOTS), np.float32)
        lensp = np.zeros((GSLOTS, NGROUPS), np.float32)
        for sb in range(NSB):
            tsb, nch = t_sbs[sb], nchs[sb]
            ko, no = plan["kt_offs"][sb], plan["kn_offs"][sb]
            g, isb = sb // GROUP_SBS, sb % GROUP_SBS
            for ssb in range(SB):
                s = sb * SB + ssb
                b = int(bmap[s, c])
                ktp[:, ko + ssb * tsb: ko + (ssb + 1) * tsb] = keys[b, :tsb, :].T
                o_ = no
                for ch, cl in enumerate(plan["kn_cls"][sb]):
                    view = knp[o_:o_ + cl * SB * H].reshape(cl, SB, H)
                    view[:, ssb, :] = keys[b, 128 * ch:128 * ch + cl, :]
                    o_ += cl * SB * H
                qtp[:, s] = query[b]
                lensp[16 * isb + ssb, g] = keys_length[b]
        # per-slot fused first-layer weights: Ws = Wk + diag(q_s) Wqk
        wsp = (Wk[:, None, :] + qtp[:, :, None] * Wqk[:, None, :])  # [H,SLOTS,H1]
        wsp = wsp.reshape(H, SLOTS * H1).astype(bfl)
        # bf16 const bundle: [qt | wq | wf(x2 bands) | w2]
        bbp = np.zeros((H, SLOTS + H1 + 1 + H2), bfl)
        bbp[:, 0:SLOTS] = qtp
        bbp[:, SLOTS:SLOTS + H1] = Wq
        bbp[0:H2, SLOTS + H1] = Wf[:, 0]
        bbp[64:64 + H2, SLOTS + H1] = Wf[:, 0]
        bbp[0:H1, SLOTS + H1 + 1:] = W2
        # f32 const bundle: [b1 | b2(x2 bands) | lens]
        fbp = np.zeros((128, 2 + NGROUPS), np.float32)
        fbp[0:H1, 0] = b1
        fbp[0:H2, 1] = b2
        fbp[64:64 + H2, 1] = b2
        fbp[0:GSLOTS, 2:] = lensp
        in_maps.append({"keysT": ktp, "knat": knp, "bb": bbp, "fb": fbp,
                        "ws": wsp})
    return in_maps


def build_program(plan):
    nc = bacc.Bacc("TRN2", num_devices=NC)
    ins = [
        nc.dram_tensor("keysT", [H, plan["kt_w"]], BF16, kind="ExternalInput").ap(),
        nc.dram_tensor("knat", [plan["kn_w"]], BF16, kind="ExternalInput").ap(),
        nc.dram_tensor("bb", [H, SLOTS + H1 + 1 + H2], BF16,
                       kind="ExternalInput").ap(),
        nc.dram_tensor("fb", [128, 2 + NGROUPS], F32, kind="ExternalInput").ap(),
        nc.dram_tensor("ws", [H, SLOTS * H1], BF16, kind="ExternalInput").ap(),
    ]
    outs = [nc.dram_tensor("outN", [SLOTS, H], F32, kind="ExternalOutput").ap()]
    with tile.TileContext(nc) as tc:
        with ExitStack() as ctx:
            build_body(ctx, tc, outs, ins, plan)
    nc.compile()
    return nc


last_results = None  # stash for external profiling/analysis


def kernel(query, keys, keys_length, W1, b1, W2, b2, Wf, bf):
    global last_results
    from concourse.bass_utils import run_bass_kernel_spmd
    query = np.asarray(query, np.float32)
    keys = np.asarray(keys, np.float32)
    keys_length = np.asarray(keys_length)
    plan = make_plan(keys_length)
    in_maps = pack_inputs(query, keys, keys_length, np.asarray(W1, np.float32),
                          np.asarray(b1, np.float32), np.asarray(W2, np.float32),
                          np.asarray(b2, np.float32), np.asarray(Wf, np.float32),
                          np.asarray(bf, np.float32), plan)
    nc = build_program(plan)
    trace = bool(int(os.environ.get("BASS_KERNEL_TRACE", "0")))
    res = run_bass_kernel_spmd(nc, in_maps, core_ids=list(range(NC)), trace=trace)
    last_results = res
    globals()["last_nc"] = nc
    if trace and res.exec_time_ns is not None:
        print(f"HW exec time: {res.exec_time_ns} ns")
    out = np.zeros((B, H), np.float32)
    bmap = plan["bmap"]
    rows = np.array([GSLOTS * (s // GSLOTS)
                     + _out_row_in_group((s // SB) % GROUP_SBS, s % SB)
                     for s in range(SLOTS)])
    for c in range(NC):
        outN = res.results[c]["outN"]  # [SLOTS, H]
        out[bmap[:, c]] = outN[rows]
    return out



# revision 47
# speedup vs baseline: 1.1235x; 1.0243x over previous
"""DIN-attention Trainium2 kernel.

out[b] = softmax_t(MLP(concat[q, k, q-k, q*k]) / sqrt(H), mask=t<len_b) @ keys[b]

Strategy (8-core data parallel over B, one shared SPMD program):
- Host sorts b by keys_length, deals round-robin to cores -> per-core slot s
  holds similar lengths on every core; per 16-slot sub-block, work is
  truncated to the sub-block max length (halves all work in expectation).
- MLP decomposition: din@W1 = k@(Wk + diag(q)Wqk) + (Wq^T q + b1) with
  Wk=W1b-W1c, Wqk=W1d, Wq=W1a+W1c; the per-slot fused weight
  Ws = Wk + diag(q_s)Wqk is packed on the host, so m1 is one matmul per
  slot; the per-slot bias (Wq^T q_s + b1) enters the PSUM accumulation
  group through a small K=ns+1 selector matmul pinned to PE row-group 3
  (tile_position=(96,0)), which runs concurrently with the m2 pair
  (K=80, row-groups 0-2) on disjoint PE sub-arrays.
- m2 chunks are emitted in pairs packed at col-groups (0,0)/(0,64) for
  2x PE concurrency; m3 is a single M=1 matmul per chunk packed at PSUM
  partition groups {0,32,64,96} via tile_position, bank-batched.
- Deferred-emission software pipeline (bias/relu1 -> m2 pair/relu2 -> m3
  banks) keeps every matmul's dependencies satisfied before it reaches the
  PE queue head; group g's transpose + output contraction are emitted
  inside group g+1's first sub-block.
- Output contraction runs attn columns (PE-transposed, bf16) as M=1
  stationaries against native-layout bf16 keys with tile_position cycling
  (4x concurrency), accumulated across 128-row chunks over a pre-zeroed
  PSUM bank.
"""

import os
import sys
from contextlib import ExitStack

for _p in ("/opt/trn_rl_repo",):
    if _p not in sys.path:
        sys.path.insert(0, _p)

os.environ.setdefault("CONCOURSE_ENABLE_LDW_OPT", "false")

import numpy as np
import ml_dtypes

import concourse.bass as bass
import concourse.tile as tile
from concourse import bacc, mybir
from concourse.masks import make_identity

F32 = mybir.dt.float32
BF16 = mybir.dt.bfloat16
A = mybir.AluOpType
AF = mybir.ActivationFunctionType

B, T, H = 2048, 200, 128
H1, H2 = 80, 40
NC = 8
SLOTS = B // NC          # 256 slots per core
SB = 16                  # slots per sub-block
NSB = SLOTS // SB        # 16 sub-blocks per core
GROUP_SBS = 4            # sub-blocks per softmax group
NGROUPS = NSB // GROUP_SBS
GSLOTS = GROUP_SBS * SB  # 64 slots per group
SCALE = float(1.0 / np.sqrt(np.float32(H)))
NEG = -1e9


def _roundup(x, m):
    return ((int(x) + m - 1) // m) * m


def _ns_of(tsb):
    # uniform power-of-two slots per chunk with ns*tsb <= 512
    for ns in (16, 8, 4, 2):
        if ns * tsb <= 512:
            return ns
    return 1


def make_plan(keys_length):
    """Global plan shared by all cores: slot assignment + per-sub-block T."""
    order = np.argsort(keys_length, kind="stable")
    bmap = order.reshape(SLOTS, NC)          # [slot, core] -> b
    t_sbs = []
    for sb in range(NSB):
        lens = np.asarray(keys_length)[bmap[sb * SB:(sb + 1) * SB]]
        m = int(lens.max())
        t_sbs.append(min(T, _roundup(m, 8)))
    nchs = [max(1, -(-t // 128)) for t in t_sbs]
    ns_sbs = [_ns_of(t) for t in t_sbs]
    kt_offs, off = [], 0
    for t in t_sbs:
        kt_offs.append(off)
        off += SB * t
    kt_w = off
    # kn: flat, per sub-block, grouped by 128-row chunk; only valid key rows
    # are stored/transferred (cl rows per chunk) to cut HBM traffic.
    kn_cls = [[min(128, t - 128 * c) for c in range(nchs[i])]
              for i, t in enumerate(t_sbs)]
    kn_offs, off = [], 0
    for sb in range(NSB):
        kn_offs.append(off)
        off += sum(kn_cls[sb]) * SB * H
    kn_w = off
    tgs = [max(t_sbs[g * GROUP_SBS:(g + 1) * GROUP_SBS]) for g in range(NGROUPS)]
    # canonical chunk order (must match build_body's emission loops)
    chunks = []
    for sb in range(NSB):
        ns = ns_sbs[sb]
        for ci in range(SB // ns):
            chunks.append((sb, ci, ci * ns, ns, t_sbs[sb]))
    tsb_vals = sorted(set(t_sbs))
    return dict(bmap=bmap, t_sbs=t_sbs, nchs=nchs, ns_sbs=ns_sbs,
                kt_offs=kt_offs, kt_w=kt_w, kn_offs=kn_offs, kn_w=kn_w,
                kn_cls=kn_cls, tgs=tgs, chunks=chunks, tsb_vals=tsb_vals)


def _out_row_in_group(isb, ssb):
    # row of slot ssb (of sub-block isb) within the group's packed out DMA:
    # traversal order (pgroup, isb, colblk)
    return (ssb % 4) * 16 + isb * 4 + ssb // 4


SECTION_MARKS = []


def _mark(nc, label):
    SECTION_MARKS.append((len(nc.inst_map), label))


def build_body(ctx, tc, outs, ins, plan):
    nc = tc.nc
    SECTION_MARKS.clear()
    keysT_d, knat_d, bb_d, fb_d, ws_d = ins
    out_d, = outs
    t_sbs, nchs, ns_sbs = plan["t_sbs"], plan["nchs"], plan["ns_sbs"]
    kt_offs, kn_offs, tgs = plan["kt_offs"], plan["kn_offs"], plan["tgs"]
    kn_cls = plan["kn_cls"]

    singles = ctx.enter_context(tc.tile_pool(name="singles", bufs=1))
    kt_pool = ctx.enter_context(tc.tile_pool(name="kt", bufs=2))
    kn_pool = ctx.enter_context(tc.tile_pool(name="kn", bufs=2 * GROUP_SBS))
    ws_pool = ctx.enter_context(tc.tile_pool(name="ws", bufs=3))
    h1_pool = ctx.enter_context(tc.tile_pool(name="h1", bufs=8))
    h2_pool = ctx.enter_context(tc.tile_pool(name="h2", bufs=8))
    scr_pool = ctx.enter_context(tc.tile_pool(name="scr", bufs=6))
    osc_pool = ctx.enter_context(tc.tile_pool(name="osc", bufs=3))
    grp_pool = ctx.enter_context(tc.tile_pool(name="grp", bufs=2))
    at_pool = ctx.enter_context(tc.tile_pool(name="at", bufs=4))
    ps1_pool = ctx.enter_context(tc.tile_pool(name="ps1", bufs=2, space="PSUM"))
    pst_pool = ctx.enter_context(tc.tile_pool(name="pst", bufs=1, space="PSUM"))
    ps2_pool = ctx.enter_context(tc.tile_pool(name="ps2", bufs=2, space="PSUM"))
    psper_pool = ctx.enter_context(tc.tile_pool(name="psper", bufs=1, space="PSUM"))

    # ---- first kt group + first sub-block weights: prefetch before the
    # constants so the opening m1 isn't starved behind small DMAs ----
    ktgs = {}
    kt_pool_ref = kt_pool

    def emit_ktg(g, split=False):
        if g >= NGROUPS or g in ktgs:
            return
        _mark(nc, 'dma_kt')
        kt0_ = kt_offs[g * GROUP_SBS]
        ktg_w = sum(SB * t_sbs[g * GROUP_SBS + i] for i in range(GROUP_SBS))
        t = kt_pool_ref.tile([H, ktg_w], BF16, tag="kt", name=f"ktg_{g}")
        if split:
            # three transfers: sb0 | sb1 | sb2+sb3, so the opening m1 can
            # begin as soon as the first (small) slice has landed
            w0 = SB * t_sbs[g * GROUP_SBS]
            w1 = w0 + SB * t_sbs[g * GROUP_SBS + 1]
            nc.sync.dma_start(t[:, 0:w0], keysT_d[:, kt0_:kt0_ + w0])
            nc.sync.dma_start(t[:, w0:w1], keysT_d[:, kt0_ + w0:kt0_ + w1])
            nc.sync.dma_start(t[:, w1:ktg_w],
                              keysT_d[:, kt0_ + w1:kt0_ + ktg_w])
        else:
            nc.sync.dma_start(t[:], keysT_d[:, kt0_:kt0_ + ktg_w])
        ktgs[g] = t

    GORDER = [1, 2, 3, 0]
    SB_ORDER = [g_ * GROUP_SBS + i_ for g_ in GORDER for i_ in range(GROUP_SBS)]
    # startup order matters: each DMA trigger occupies the queue ~0.6-0.9us,
    # so the first m1's dependencies (kt sb0 slice, const bundles, wss0) are
    # triggered FIRST, in 4 transfers total.
    sbf = SB_ORDER[0]
    kt0_ = kt_offs[sbf]
    w0_ = SB * t_sbs[sbf]
    ktg_w0 = sum(SB * t_sbs[sbf + i] for i in range(GROUP_SBS))
    ktg0 = kt_pool.tile([H, ktg_w0], BF16, tag="kt", name=f"ktg_{GORDER[0]}")
    nc.sync.dma_start(ktg0[:, 0:w0_], keysT_d[:, kt0_:kt0_ + w0_])
    wss_first = ws_pool.tile([H, SB * H1], BF16, tag="wss", name=f"wss_{sbf}")
    nc.sync.dma_start(wss_first[:], ws_d[:, sbf * SB * H1:(sbf + 1) * SB * H1])
    # bb: [qt | wq | wf2 | w2] bf16 bundle, fb: [b1 | b2(x2) | lens] f32 bundle
    bb = singles.tile([H, SLOTS + H1 + 1 + H2], BF16, name="bb")
    nc.sync.dma_start(bb[:], bb_d)
    fb = singles.tile([128, 2 + NGROUPS], F32, name="fb")
    nc.sync.dma_start(fb[:], fb_d)
    qt = bb[:, 0:SLOTS]
    wq = bb[:, SLOTS:SLOTS + H1]
    wf2 = bb[:, SLOTS + H1:SLOTS + H1 + 1]
    w2 = bb[0:H1, SLOTS + H1 + 1:SLOTS + H1 + 1 + H2]
    b1c = fb[0:H1, 0:1]
    b2c = fb[0:H2, 1:2]
    b2c2 = fb[0:64 + H2, 1:2]
    lens = fb[0:GSLOTS, 2:2 + NGROUPS]
    wss_pre = {sbf: wss_first}
    t_ = ws_pool.tile([H, SB * H1], BF16, tag="wss", name=f"wss_{sbf + 1}")
    nc.sync.dma_start(t_[:], ws_d[:, (sbf + 1) * SB * H1:(sbf + 2) * SB * H1])
    wss_pre[sbf + 1] = t_
    # rest of the first group's keysT
    w1_ = w0_ + SB * t_sbs[sbf + 1]
    nc.sync.dma_start(ktg0[:, w0_:w1_], keysT_d[:, kt0_ + w0_:kt0_ + w1_])
    nc.sync.dma_start(ktg0[:, w1_:ktg_w0], keysT_d[:, kt0_ + w1_:kt0_ + ktg_w0])
    ktgs[GORDER[0]] = ktg0
    iota = singles.tile([128, T], F32, name="iota")
    nc.gpsimd.iota(iota[:], pattern=[[1, T]], base=0, channel_multiplier=0,
                   allow_small_or_imprecise_dtypes=True)
    identb = singles.tile([128, 128], BF16, name="identb")
    make_identity(nc, identb[:])
    zeros1 = singles.tile([1, 128], BF16, name="zeros1")
    nc.vector.memset(zeros1[:], 0.0)
    dummy512 = singles.tile([1, 512], BF16, name="dummy512")
    nc.vector.memset(dummy512[:], 0.0)
    # per-group per-slot length masks (t < len), on the otherwise-idle gpsimd
    masks = []
    for g in range(NGROUPS):
        tg = tgs[g]
        mk = singles.tile([GSLOTS, tg], F32, name=f"mask_{g}")
        nc.gpsimd.tensor_scalar(mk[:], iota[0:GSLOTS, 0:tg],
                                fb[0:GSLOTS, 2 + g:3 + g],
                                None, op0=A.is_lt)
        masks.append(mk)
    # persistent, one-time-zeroed PSUM banks
    pss_t = [psper_pool.tile([128, 512], F32, tag=f"pssp{i}", name=f"pssp{i}")
             for i in range(2)]
    pso_t = [psper_pool.tile([128, 512], F32, tag="psop0", name="psop0")] * 2
    for t_ in pss_t + pso_t[:1]:
        nc.tensor.matmul(t_[:], zeros1[:], dummy512[:], start=True, stop=True)


    bb_pitch = bb[:].ap[0][0]
    bb_off = bb[:].offset
    par = [0]      # parity counter for DVE/ACT copy balancing
    bankctr = [0]  # m3 scores-bank alternation
    ckctr = [0]    # global chunk counter

    # deferred-emission pipeline queues. Each m2q/m3q entry is (mm, post):
    # the mm parts of a pair/bank are emitted NEWEST-FIRST so the first
    # matmul's semaphore wait subsumes the older ones — the rest arrive at
    # the PE wait-free and overlap via their disjoint tile_position groups
    # (the same reason the final stage's shared-dep matmuls overlap).
    m2q, m3q = [], []
    finq = []
    zsums = {}
    trans_prev = [None]
    final_prev = [None]

    def pump_m3(force=False):
        while m3q:
            key = m3q[0][0]
            nbank = sum(1 for k, _ in m3q if k == key)
            rest = len(m3q) - nbank
            if rest < 6 and not force:
                return
            units = [m3q.pop(0)[1] for _ in range(nbank)]
            for u in reversed(units):
                u[0]()
            for u in units:
                u[1]()

    def pump(force=False):
        while len(m2q) >= (2 if force else 3):
            ua, ub = m2q.pop(0), m2q.pop(0)
            ps2 = ps2_pool.tile([128, 512], F32, tag="ps2",
                                name=f"ps2p_{ckctr[0]}_{len(m2q)}")
            ub[0](ps2, 64)
            ua[0](ps2, 0)
            maxc = max(ua[2], ub[2])
            _mark(nc, 'relu2')
            h2 = h2_pool.tile([64 + H2, 512], BF16, tag="h2",
                              name=f"h2p_{ckctr[0]}_{len(m2q)}")
            if par[0] % 2 == 0:
                nc.scalar.activation(h2[:, 0:maxc], ps2[0:64 + H2, 0:maxc],
                                     AF.Relu, bias=b2c2, scale=1.0)
            else:
                nc.vector.tensor_scalar(h2[:, 0:maxc], ps2[0:64 + H2, 0:maxc],
                                        b2c2, 0.0, op0=A.add, op1=A.max)
            par[0] += 1
            ua[1](h2, 0)
            ub[1](h2, 64)
        if force and m2q:
            ps2 = ps2_pool.tile([128, 512], F32, tag="ps2", name="ps2last")
            u = m2q.pop(0)
            u[0](ps2, 0)
            _mark(nc, 'relu2')
            h2 = h2_pool.tile([64 + H2, 512], BF16, tag="h2", name="h2last")
            nc.scalar.activation(h2[0:H2, 0:u[2]], ps2[0:H2, 0:u[2]],
                                 AF.Relu, bias=b2c, scale=1.0)
            u[1](h2, 0)
        pump_m3(force)

    for gi, g in enumerate(GORDER):
        tg = tgs[g]
        scores = grp_pool.tile([GSLOTS, tg], F32, tag="scores", name=f"scores_g{g}")
        _mark(nc, 'memset')
        nc.gpsimd.memset(scores[:], 0.0)
        emit_ktg(g)
        ktg = ktgs.pop(g)
        kt0 = kt_offs[g * GROUP_SBS]
        knats = {}
        for isb in range(GROUP_SBS):
            sb = g * GROUP_SBS + isb
            tsb, nch, ns = t_sbs[sb], nchs[sb], ns_sbs[sb]
            nchunks = SB // ns
            kto = kt_offs[sb] - kt0
            wss = wss_pre.pop(sb)
            # prefetch the fused weights two sub-blocks ahead so the m1
            # stream never waits on the ws DMA
            pos_ = SB_ORDER.index(sb)
            for sbn in SB_ORDER[pos_ + 1:pos_ + 3]:
                if sbn not in wss_pre:
                    _mark(nc, 'dma_ws')
                    t_ = ws_pool.tile([H, SB * H1], BF16, tag="wss",
                                      name=f"wss_{sbn}")
                    nc.sync.dma_start(t_[:],
                                      ws_d[:, sbn * SB * H1:(sbn + 1) * SB * H1])
                    wss_pre[sbn] = t_
            _mark(nc, 'dma_kn')
            kn = kn_pool.tile([128, SB * nch * 128], BF16, tag="kn", name=f"kn_{sb}")
            o_ = kn_offs[sb]
            for c_, cl_ in enumerate(kn_cls[sb]):
                src = bass.AP(tensor=knat_d.tensor, offset=o_,
                              ap=[[SB * H, cl_], [1, SB * H]])
                nc.sync.dma_start(kn[0:cl_, c_ * SB * H:(c_ + 1) * SB * H], src)
                o_ += cl_ * SB * H
            knats[sb] = (kn, nch)

            for bank in range(-(-nchunks // 4)):
                cis = range(4 * bank, min(nchunks, 4 * bank + 4))
                cols = ns * tsb
                for ci in cis:
                    s0 = ci * ns
                    coff = kto + s0 * tsb
                    ckctr[0] += 1
                    _mark(nc, 'm1')
                    ps1 = ps1_pool.tile([H1, cols], F32, tag="ps1",
                                        name=f"ps1_{sb}_{ci}")
                    for j in range(ns):
                        si = (s0 + j) * H1
                        nc.tensor.matmul(ps1[:, j * tsb:(j + 1) * tsb],
                                         wss[:, si:si + H1],
                                         ktg[:, coff + j * tsb:coff + (j + 1) * tsb],
                                         start=(j == 0), stop=False,
                                         skip_group_check=True)
                    qview = bass.AP(tensor=bb[:].tensor,
                                    offset=bb_off + sb * SB + s0,
                                    ap=[[bb_pitch, H], [1, ns], [0, tsb]])
                    nc.tensor.matmul(ps1[:, 0:cols], wq, qview,
                                     start=False, stop=True,
                                     skip_group_check=True)
                    _mark(nc, 'relu1')
                    h1 = h1_pool.tile([H1, cols], BF16, tag="h1",
                                      name=f"h1_{sb}_{ci}")
                    if ckctr[0] % 3 == 0:
                        nc.scalar.activation(h1[:], ps1[:], AF.Relu,
                                             bias=b1c, scale=1.0)
                    else:
                        nc.vector.tensor_scalar(h1[:], ps1[:], b1c, 0.0,
                                                op0=A.add, op1=A.max)

                    def m2_mm(ps2, base, cols=cols, h1=h1):
                        _mark(nc, 'm2')
                        nc.tensor.matmul(ps2[base:base + H2, 0:cols], w2,
                                         h1[:], start=True, stop=True,
                                         tile_position=(0, base),
                                         skip_group_check=True)

                    def m2_post(h2, rowbase, sb=sb, isb=isb, bank=bank, ci=ci,
                                cols=cols, tsb=tsb, ns=ns,
                                last=(ci == cis[-1]), npg=len(cis),
                                scores=scores):
                        def m3_mm():
                            _mark(nc, 'm3')
                            pg = ci % 4
                            pss = pss_t[bankctr[0] % 2]
                            nc.tensor.matmul(pss[32 * pg:32 * pg + 1, 0:cols],
                                             bb[rowbase:rowbase + H2,
                                                SLOTS + H1:SLOTS + H1 + 1],
                                             h2[rowbase:rowbase + H2, 0:cols],
                                             start=True, stop=True,
                                             tile_position=(rowbase, 32 * pg),
                                             skip_group_check=True)

                        def m3_post():
                            if not last:
                                return
                            pss = pss_t[bankctr[0] % 2]
                            bankctr[0] += 1
                            _mark(nc, 'scr_copy')
                            scratch = scr_pool.tile([128, 512], F32, tag="scr",
                                                    name=f"scr_{sb}_{bank}")
                            nc.scalar.activation(scratch[:, 0:cols],
                                                 pss[:, 0:cols], AF.Exp,
                                                 bias=0.0, scale=SCALE)
                            _mark(nc, 'redis_s')
                            scr_pitch = scratch[:].ap[0][0]
                            src = bass.AP(tensor=scratch[:].tensor,
                                          offset=scratch[:].offset,
                                          ap=[[32 * scr_pitch, npg],
                                              [tsb, ns], [1, tsb]])
                            r0 = 16 * isb + 4 * bank * ns
                            nc.gpsimd.dma_start(
                                scores[r0:r0 + npg * ns, 0:tsb], src)

                        m3q.append(((sb, bank), (m3_mm, m3_post)))

                    m2q.append((m2_mm, m2_post, cols))
                    pump()
            if isb == 0 and finq:
                finq.pop(0)()
            if isb == 1:
                if finq:
                    finq.pop(0)()
                if gi + 1 < NGROUPS:
                    emit_ktg(GORDER[gi + 1])
            if isb == 2:
                if trans_prev[0] is not None:
                    trans_prev[0]()
                    trans_prev[0] = None
                if final_prev[0] is not None:
                    finq.extend(final_prev[0]())
                    final_prev[0] = None
                for _ in range(2):
                    if finq:
                        finq.pop(0)()
            if isb == 3:
                for _ in range(2):
                    if finq:
                        finq.pop(0)()
        pump(force=True)

        atts = []

        def trans_unit(g=g, tg=tg, scores=scores, atts=atts):
            # softmax mostly on the otherwise-idle gpsimd so DVE/ACT queues
            # stay clear for the relu stream
            _mark(nc, 'softmax')
            pm = grp_pool.tile([GSLOTS, tg], F32, tag="pm", name=f"pm_{g}")
            nc.vector.tensor_tensor(pm[:], scores[:], masks[g][:], op=A.mult)
            zsum = grp_pool.tile([GSLOTS, 1], F32, tag="zsum", name=f"zsum_{g}")
            nc.vector.reduce_sum(zsum[:], pm[:], axis=mybir.AxisListType.X)
            rz = grp_pool.tile([GSLOTS, 1], F32, tag="rz", name=f"rz_{g}")
            nc.vector.reciprocal(rz[:], zsum[:])
            attnb = grp_pool.tile([GSLOTS, tg], BF16, tag="attnb",
                                  name=f"attnb_{g}")
            nc.vector.tensor_scalar_mul(attnb[:], pm[:], rz[:, 0:1])
            _mark(nc, 'transpose')
            for c in range(-(-tg // 128)):
                cl = min(128, tg - 128 * c)
                ps_t = pst_pool.tile([cl, GSLOTS], BF16, tag="pst",
                                     name=f"pst_{g}_{c}")
                nc.tensor.transpose(ps_t[:], attnb[:, 128 * c:128 * c + cl],
                                    identb[0:GSLOTS, 0:GSLOTS])
                at = at_pool.tile([cl, GSLOTS], BF16, tag="at", name=f"at_{g}_{c}")
                nc.vector.tensor_copy(at[:], ps_t[:])
                atts.append(at)

        def make_final_units(g=g, knats=knats, atts=atts):
            oscr_box = [None]

            def f_sb(isb):
                def run():
                    _mark(nc, 'final')
                    if oscr_box[0] is None:
                        oscr_box[0] = osc_pool.tile(
                            [128, GROUP_SBS * 512], F32, tag="oscr",
                            name=f"oscr_{g}")
                    oscr = oscr_box[0]
                    sb = g * GROUP_SBS + isb
                    tsb = t_sbs[sb]
                    kn, nch = knats[sb]
                    ps_o = pso_t[sb % 2]
                    if nch == 1:
                        for ssb in range(SB):
                            r = 16 * isb + ssb
                            cl = tsb
                            blk = ssb * 128
                            nc.tensor.matmul(
                                ps_o[32 * (ssb % 4):32 * (ssb % 4) + 1,
                                     128 * (ssb // 4):128 * (ssb // 4) + 128],
                                atts[0][0:cl, r:r + 1], kn[0:cl, blk:blk + 128],
                                start=True, stop=True,
                                tile_position=(0, 32 * (ssb % 4)),
                                skip_group_check=True)
                    else:
                        nc.tensor.matmul(ps_o[:], zeros1[:], dummy512[:],
                                         start=True, stop=False,
                                         skip_group_check=True)
                        for c in range(nch):
                            cl = min(128, tsb - 128 * c)
                            for ssb in range(SB):
                                r = 16 * isb + ssb
                                blk = c * SB * H + ssb * 128
                                nc.tensor.matmul(
                                    ps_o[32 * (ssb % 4):32 * (ssb % 4) + 1,
                                         128 * (ssb // 4):128 * (ssb // 4) + 128],
                                    atts[c][0:cl, r:r + 1],
                                    kn[0:cl, blk:blk + 128],
                                    start=False, stop=(c == nch - 1),
                                    tile_position=(0, 32 * (ssb % 4)),
                                    skip_group_check=True)
                    _mark(nc, 'oscr_copy')
                    nc.vector.tensor_copy(oscr[:, isb * 512:isb * 512 + 512],
                                          ps_o[:])
                return run

            def f_out():
                _mark(nc, 'out_dma')
                oscr = oscr_box[0]
                os_pitch = oscr[:].ap[0][0]
                src = bass.AP(tensor=oscr[:].tensor, offset=oscr[:].offset,
                              ap=[[32 * os_pitch, 4], [512, 4], [128, 4],
                                  [1, 128]])
                nc.sync.dma_start(out_d[g * GSLOTS:(g + 1) * GSLOTS, :], src)

            return [f_sb(0), f_sb(1), f_sb(2), f_sb(3), f_out]

        trans_prev[0] = trans_unit
        final_prev[0] = make_final_units
    trans_prev[0]()
    for u in final_prev[0]():
        u()
    while finq:
        finq.pop(0)()


def pack_inputs(query, keys, keys_length, W1, b1, W2, b2, Wf, bf, plan):
    """Build the 8 per-core input maps."""
    bmap, t_sbs, nchs = plan["bmap"], plan["t_sbs"], plan["nchs"]
    kt_w, kn_w = plan["kt_w"], plan["kn_w"]
    Wq = (W1[0:H] + W1[2 * H:3 * H]).astype(np.float32)
    Wk = (W1[H:2 * H] - W1[2 * H:3 * H]).astype(np.float32)
    Wqk = W1[3 * H:4 * H].astype(np.float32)
    bfl = ml_dtypes.bfloat16
    in_maps = []
    for c in range(NC):
        ktp = np.zeros((H, kt_w), bfl)
        knp = np.zeros((kn_w,), bfl)
        qtp = np.zeros((H, SLOTS), np.float32)
        lensp = np.zeros((GSLOTS, NGROUPS), np.float32)
        for sb in range(NSB):
            tsb, nch = t_sbs[sb], nchs[sb]
            ko, no = plan["kt_offs"][sb], plan["kn_offs"][sb]
            g, isb = sb // GROUP_SBS, sb % GROUP_SBS
            for ssb in range(SB):
                s = sb * SB + ssb
                b = int(bmap[s, c])
                ktp[:, ko + ssb * tsb: ko + (ssb + 1) * tsb] = keys[b, :tsb, :].T
                o_ = no
                for ch, cl in enumerate(plan["kn_cls"][sb]):
                    view = knp[o_:o_ + cl * SB * H].reshape(cl, SB, H)
                    view[:, ssb, :] = keys[b, 128 * ch:128 * ch + cl, :]
                    o_ += cl * SB * H
                qtp[:, s] = query[b]
                lensp[16 * isb + ssb, g] = keys_length[b]
        # per-slot fused first-layer weights: Ws = Wk + diag(q_s) Wqk
        wsp = (Wk[:, None, :] + qtp[:, :, None] * Wqk[:, None, :])  # [H,SLOTS,H1]
        wsp = wsp.reshape(H, SLOTS * H1).astype(bfl)
        # bf16 const bundle: [qt | wq | wf(x2 bands) | w2]
        bbp = np.zeros((H, SLOTS + H1 + 1 + H2), bfl)
        bbp[:, 0:SLOTS] = qtp
        bbp[:, SLOTS:SLOTS + H1] = Wq
        bbp[0:H2, SLOTS + H1] = Wf[:, 0]
        bbp[64:64 + H2, SLOTS + H1] = Wf[:, 0]
        bbp[0:H1, SLOTS + H1 + 1:] = W2
        # f32 const bundle: [b1 | b2(x2 bands) | lens]
        fbp = np.zeros((128, 2 + NGROUPS), np.float32)
        fbp[0:H1, 0] = b1
        fbp[0:H2, 1] = b2
        fbp[64:64 + H2, 1] = b2
        fbp[0:GSLOTS, 2:] = lensp
        in_maps.append({"keysT": ktp, "knat": knp, "bb": bbp, "fb": fbp,
                        "ws": wsp})
    return in_maps


def build_program(plan):
    nc = bacc.Bacc("TRN2", num_devices=NC)
    ins = [
        nc.dram_tensor("keysT", [H, plan["kt_w"]], BF16, kind="ExternalInput").ap(),
        nc.dram_tensor("knat", [plan["kn_w"]], BF16, kind="ExternalInput").ap(),
        nc.dram_tensor("bb", [H, SLOTS + H1 + 1 + H2], BF16,
                       kind="ExternalInput").ap(),
        nc.dram_tensor("fb", [128, 2 + NGROUPS], F32, kind="ExternalInput").ap(),
        nc.dram_tensor("ws", [H, SLOTS * H1], BF16, kind="ExternalInput").ap(),
    ]
    outs = [nc.dram_tensor("outN", [SLOTS, H], F32, kind="ExternalOutput").ap()]
    with tile.TileContext(nc) as tc:
        with ExitStack() as ctx:
            build_body(ctx, tc, outs, ins, plan)
    nc.compile()
    return nc


last_results = None  # stash for external profiling/analysis


def kernel(query, keys, keys_length, W1, b1, W2, b2, Wf, bf):
    global last_results
    from concourse.bass_utils import run_bass_kernel_spmd
    query = np.asarray(query, np.float32)
    keys = np.asarray(keys, np.float32)
    keys_length = np.asarray(keys_length)
    plan = make_plan(keys_length)
    in_maps = pack_inputs(query, keys, keys_length, np.asarray(W1, np.float32),
                          np.asarray(b1, np.float32), np.asarray(W2, np.float32),
                          np.asarray(b2, np.float32), np.asarray(Wf, np.float32),
                          np.asarray(bf, np.float32), plan)
    nc = build_program(plan)
    trace = bool(int(os.environ.get("BASS_KERNEL_TRACE", "0")))
    res = run_bass_kernel_spmd(nc, in_maps, core_ids=list(range(NC)), trace=trace)
    last_results = res
    globals()["last_nc"] = nc
    if trace and res.exec_time_ns is not None:
        print(f"HW exec time: {res.exec_time_ns} ns")
    out = np.zeros((B, H), np.float32)
    bmap = plan["bmap"]
    rows = np.array([GSLOTS * (s // GSLOTS)
                     + _out_row_in_group((s // SB) % GROUP_SBS, s % SB)
                     for s in range(SLOTS)])
    for c in range(NC):
        outN = res.results[c]["outN"]  # [SLOTS, H]
        out[bmap[:, c]] = outN[rows]
    return out

